# revision 25
# baseline (speedup 1.0000x reference)
"""Trainium2 Bass kernel for the CG tensor-product iteration (nn_CGIteration).

Computation per l (L_MAX=3, K=256, N=2048):
    tp_l   = concat_{(l1,l2) in PATHS[l]} einsum('abm,nak,nbk->nmk', CG, f1_l1, f2_l2)
    out_l  = f1_l + (tp_l @ U_l) @ W_l

Sharding: data-parallel over N across 8 cores (N/8 = 256 per core).

Device-side layout: channels-on-partitions.  Host pre-transposes f1/f2 to
[k, a, n] per core, precomputes U'_l = U_l @ W_l (fp64) and the residual add +
final transpose happen on host.  On-chip per core:
  1. products P_ab[k, n] = f1T[:, a, :] * f2T[:, b, :]          (DVE/GpSimd, fp16)
  2. CG FMAs  tpT[(p,k), (m, n)] += cg * P_ab                   (DVE + ACT, fp16)
     - first write to a tp slice is a scaled copy on the Scalar engine
     - (m3, -m3) partner terms of even paths merge into one 2-point op
  3. U' matmul: psum[j, (m,n)] += U'[(p,k), j].T @ tpT           (PE, fp16 -> fp32)
     The two k-half phases are written to DRAM as separate fp32 partials
     and summed on the host together with the residual.
"""

import os
from contextlib import ExitStack
from math import factorial

import numpy as np

import concourse.bass as bass
import concourse.mybir as mybir
from concourse import bacc, tile
from concourse.bass_utils import run_bass_kernel_spmd

L_MAX = 3
K = 256
N = 2048
NCORES = 8
NP_ = N // NCORES          # samples per core (256)
P = 128                    # SBUF partitions
NKC = K // P               # k-half chunks (2)
A_OFF = [0, 1, 4, 9]       # offset of each l-block among the 16 m-components
A_TOT = 16

PATHS = {l: [(l1, l2) for l1 in range(L_MAX + 1) for l2 in range(L_MAX + 1)
             if abs(l1 - l2) <= l <= l1 + l2] for l in range(L_MAX + 1)}
CIN = {l: len(PATHS[l]) * K for l in range(L_MAX + 1)}

# fraction of ops routed to GpSimd (DVE is the bottleneck engine)
GPS_PROD_FRAC = float(os.environ.get("GPS_PROD_FRAC", "0.55"))
GPS_SINGLE_FRAC = float(os.environ.get("GPS_SINGLE_FRAC", "0.85"))
GPS_MERGE_FRAC = float(os.environ.get("GPS_MERGE_FRAC", "0.0"))
USE_PAIR = os.environ.get("PAIR", "1") == "1"
USE_ACT_FIRST = os.environ.get("ACT_FIRST", "1") == "1"
USE_PAIR_PROD = os.environ.get("PAIR_PROD", "1") == "1"
KERNEL_LOOP = int(os.environ.get("KERNEL_LOOP", "1"))


def _cg_coeff(l1, m1, l2, m2, l3, m3):
    if m3 != m1 + m2:
        return 0.0
    pref = ((2 * l3 + 1) * factorial(l3 + l1 - l2) * factorial(l3 - l1 + l2)
            * factorial(l1 + l2 - l3) / factorial(l1 + l2 + l3 + 1)) ** 0.5
    pref *= (factorial(l3 + m3) * factorial(l3 - m3) * factorial(l1 - m1)
             * factorial(l1 + m1) * factorial(l2 - m2) * factorial(l2 + m2)) ** 0.5
    s = 0.0
    for k in range(0, l1 + l2 - l3 + 1):
        d = [k, l1 + l2 - l3 - k, l1 - m1 - k, l2 + m2 - k,
             l3 - l2 + m1 + k, l3 - l1 - m2 + k]
        if min(d) < 0:
            continue
        den = 1.0
        for x in d:
            den *= factorial(x)
        s += (-1.0) ** k / den
    return pref * s


def _cg_tensor(l1, l2, l3):
    out = np.zeros((2 * l1 + 1, 2 * l2 + 1, 2 * l3 + 1), dtype=np.float64)
    for m1 in range(-l1, l1 + 1):
        for m2 in range(-l2, l2 + 1):
            m3 = m1 + m2
            if -l3 <= m3 <= l3:
                out[m1 + l1, m2 + l2, m3 + l3] = _cg_coeff(l1, m1, l2, m2, l3, m3)
    return out


CG = {(l1, l2, l): _cg_tensor(l1, l2, l)
      for l in range(L_MAX + 1) for (l1, l2) in PATHS[l]}

# combo order: complete the heaviest output l first so its matmuls overlap
# with the remaining CG work
COMBO_ORDER = ([(0, 3), (1, 2), (1, 3), (2, 1), (2, 2), (2, 3), (3, 0), (3, 1),
                (3, 2), (3, 3)] + [(0, 2), (1, 1), (2, 0)] + [(0, 1), (1, 0)]
               + [(0, 0)])
# after which combo (index) each l's paths are all complete
L_READY_AT = {3: 9, 2: 12, 1: 14, 0: 15}

# ---------------------------------------------------------------------------
# Static op schedule.  Per combo: a list of "orbit" entries.  Each entry:
#   prods: [(ga, gb)] or [(ga, gb), (gpa, gpb)]   global m-component indices
#   merges: [(l, p_idx, mlo, mhi, half_of_mlo, cg)]  2-point ops (mhi > mlo)
#   singles: [(l, p_idx, m3, half, cg)]
# ---------------------------------------------------------------------------


def _build_schedule():
    sched = {}
    for (l1, l2) in COMBO_ORDER:
        # gather all nonzero CG terms of this combo: (a, b) -> [(l, p, m3, cg)]
        uses = {}
        for l in range(L_MAX + 1):
            if (l1, l2) not in PATHS[l]:
                continue
            p_idx = PATHS[l].index((l1, l2))
            cg = CG[(l1, l2, l)]
            for a in range(2 * l1 + 1):
                for b in range(2 * l2 + 1):
                    for m3 in range(2 * l + 1):
                        c = cg[a, b, m3]
                        if abs(c) > 1e-12:
                            uses.setdefault((a, b), []).append((l, p_idx, m3, float(c)))

        entries = []
        done_ab = set()
        for (a, b) in sorted(uses.keys()):
            if (a, b) in done_ab:
                continue
            pa, pb = 2 * l1 - a, 2 * l2 - b
            self_paired = (pa, pb) == (a, b)
            if self_paired or (pa, pb) not in uses or not USE_PAIR:
                done_ab.add((a, b))
                prods = [(A_OFF[l1] + a, A_OFF[l2] + b)]
                singles = [(l, p, m3, 0, c) for (l, p, m3, c) in uses[(a, b)]]
                entries.append(dict(prods=prods, merges=[], singles=singles))
                continue
            # two-product orbit
            done_ab.add((a, b))
            done_ab.add((pa, pb))
            prods = [(A_OFF[l1] + a, A_OFF[l2] + b),
                     (A_OFF[l1] + pa, A_OFF[l2] + pb)]
            merges, singles = [], []
            emitted = set()
            for half, (aa, bb) in enumerate([(a, b), (pa, pb)]):
                for (l, p, m3, c) in uses[(aa, bb)]:
                    if (l, p, m3, half) in emitted:
                        continue
                    s = (-1) ** (l1 + l2 - l)
                    pm = 2 * l - m3
                    if s == 1 and pm != m3:
                        # partner term lives on the other half with same cg
                        oh = 1 - half
                        if m3 < pm:
                            merges.append((l, p, m3, pm, half, c))
                        else:
                            merges.append((l, p, pm, m3, oh, c))
                        emitted.add((l, p, m3, half))
                        emitted.add((l, p, pm, oh))
                    elif (l, p, m3, half) not in emitted:
                        singles.append((l, p, m3, half, c))
                        emitted.add((l, p, m3, half))
            # dedupe merges (each pair appears from both halves)
            merges = sorted(set(merges))
            entries.append(dict(prods=prods, merges=merges, singles=singles))
        sched[(l1, l2)] = entries
    return sched


SCHED = _build_schedule()

DT = mybir.dt.float16
NP_DT = np.float16

_BUILT = None


def _build():
    """Build the single-core SPMD Bass program (same on all 8 cores)."""
    nc = bacc.Bacc(None, target_bir_lowering=False)

    f1t_ext = nc.declare_dram_parameter("f1t", [NKC, P, A_TOT * NP_], DT, isOutput=False)
    f2t_ext = nc.declare_dram_parameter("f2t", [NKC, P, A_TOT * NP_], DT, isOutput=False)
    n_up_chunks = sum(CIN[l] for l in range(4)) // P       # 68
    up_ext = nc.declare_dram_parameter("up", [n_up_chunks, P, K], DT, isOutput=False)
    out_ext = [nc.declare_dram_parameter(f"outT{l}", [NKC, K, (2 * l + 1) * NP_],
                                         mybir.dt.float32, isOutput=True)
               for l in range(4)]

    up_chunk_base = {}
    acc_chunks = 0
    for l in range(4):
        up_chunk_base[l] = acc_chunks
        acc_chunks += CIN[l] // P

    # engine pickers: per-slice deterministic so an accumulation chain for a
    # given tp slice stays on one engine (cross-engine RAW hops stall the
    # in-order queues)
    def make_picker(frac):
        cnt = [0.0]

        def pick():
            cnt[0] += frac
            if cnt[0] >= 1.0:
                cnt[0] -= 1.0
                return nc.gpsimd
            return nc.vector
        return pick

    prod_engine = make_picker(GPS_PROD_FRAC)

    def slice_engine(l, p_idx, m3):
        h = (l * 131 + p_idx * 31 + m3 * 7) % 100
        return nc.gpsimd if h < GPS_SINGLE_FRAC * 100 else nc.vector

    merge_engine = make_picker(GPS_MERGE_FRAC)

    with tile.TileContext(nc) as tc:
        with (
            tc.tile_pool(name="inp", bufs=1) as inp_pool,
            tc.tile_pool(name="upp", bufs=1) as up_pool,
            tc.tile_pool(name="tp", bufs=1) as tp_pool,
            tc.tile_pool(name="prod2", bufs=24) as prod2_pool,
            tc.tile_pool(name="prod1", bufs=8) as prod1_pool,
            tc.tile_pool(name="stage", bufs=4) as stage_pool,
            tc.tile_pool(name="ps", bufs=8, space="PSUM") as ps_pool,
        ):
            loop_ctx = ExitStack()
            if KERNEL_LOOP > 1:
                loop_ctx.enter_context(tc.For_i(0, KERNEL_LOOP, 1))

            up_t = up_pool.tile([P, n_up_chunks * K], DT, tag="up")
            nc.sync.dma_start(out=up_t[:], in_=up_ext.rearrange("c p k -> p c k"))

            f1_t = [inp_pool.tile([P, A_TOT * NP_], DT, tag=f"f1_{kc}", name=f"f1_{kc}")
                    for kc in range(NKC)]
            f2_t = [inp_pool.tile([P, A_TOT * NP_], DT, tag=f"f2_{kc}", name=f"f2_{kc}")
                    for kc in range(NKC)]
            for kc in range(NKC):
                nc.sync.dma_start(out=f1_t[kc][:], in_=f1t_ext[kc])
                nc.sync.dma_start(out=f2_t[kc][:], in_=f2t_ext[kc])

            RT = {l: [(i * 512, min(512, (2 * l + 1) * NP_ - i * 512))
                      for i in range(((2 * l + 1) * NP_ + 511) // 512)]
                  for l in range(4)}

            pending_evicts = []

            def emit_matmuls(l, kc, tp_t):
                flush_evicts()
                npaths = len(PATHS[l])
                for jh in range(2):
                    for ri, (r0, rw) in enumerate(RT[l]):
                        ps = ps_pool.tile([P, rw], mybir.dt.float32, tag="ps",
                                          name=f"ps_{l}_{jh}_{ri}_{kc}")
                        for pc in range(npaths):
                            chunk = up_chunk_base[l] + pc * NKC + kc
                            lhsT = up_t[:, chunk * K + jh * P: chunk * K + jh * P + P]
                            rhs = tp_t[(l, pc)][:, r0:r0 + rw]
                            nc.tensor.matmul(ps[:], lhsT, rhs,
                                             start=(pc == 0), stop=(pc == npaths - 1))
                        pending_evicts.append((ps, l, jh, ri, r0, rw, kc))

            def flush_evicts():
                for ps, l, jh, ri, r0, rw, kc in pending_evicts:
                    st = stage_pool.tile([P, rw], mybir.dt.float32, tag="stage",
                                         name=f"st_{l}_{jh}_{ri}_{kc}")
                    nc.scalar.activation(
                        st[:], ps[:], mybir.ActivationFunctionType.Copy)
                    nc.sync.dma_start(
                        out=out_ext[l][kc, jh * P:(jh + 1) * P, r0:r0 + rw],
                        in_=st[:])
                pending_evicts.clear()

            for kc in range(NKC):
                tp_t = {}
                tp3 = {}
                written = set()
                for l in range(4):
                    for p_idx in range(len(PATHS[l])):
                        t = tp_pool.tile([P, (2 * l + 1) * NP_], DT,
                                         tag=f"tp_{l}_{p_idx}", name=f"tp_{l}_{p_idx}_{kc}")
                        tp_t[(l, p_idx)] = t
                        tp3[(l, p_idx)] = t[:].rearrange("p (m n) -> p m n", n=NP_)

                for ci, (l1, l2) in enumerate(COMBO_ORDER):
                    entry_tiles = []
                    for entry in SCHED[(l1, l2)]:
                        prods = entry["prods"]
                        if len(prods) == 2:
                            (ga, gb), (gpa, gpb) = prods
                            da, db = gpa - ga, gpb - gb
                            pt = prod2_pool.tile([P, 2 * NP_], DT, tag="prod2",
                                                name=f"pp_{kc}_{l1}{l2}_{ga}_{gb}")
                            p3 = pt[:].rearrange("p (g n) -> p g n", n=NP_)
                            if USE_PAIR_PROD and da != 0 and db != 0:
                                # one 2-point-AP multiply computes both halves
                                b1 = f1_t[kc][:]
                                b2 = f2_t[kc][:]
                                in0 = bass.AP(b1.tensor, b1.offset + ga * NP_,
                                              [list(b1.ap[0]), [da * NP_, 2], [1, NP_]])
                                in1 = bass.AP(b2.tensor, b2.offset + gb * NP_,
                                              [list(b2.ap[0]), [db * NP_, 2], [1, NP_]])
                                prod_engine().tensor_mul(p3[:, 0:2, :], in0, in1)
                            else:
                                for h, (xa, xb) in enumerate(prods):
                                    prod_engine().tensor_mul(
                                        pt[:, h * NP_:(h + 1) * NP_],
                                        f1_t[kc][:, xa * NP_:(xa + 1) * NP_],
                                        f2_t[kc][:, xb * NP_:(xb + 1) * NP_])
                            halves = [pt[:, 0:NP_], pt[:, NP_:2 * NP_]]
                        else:
                            ga, gb = prods[0]
                            pt = prod1_pool.tile([P, NP_], DT, tag="prod1",
                                                name=f"pp_{kc}_{l1}{l2}_{ga}_{gb}")
                            p3 = None
                            prod_engine().tensor_mul(
                                pt[:],
                                f1_t[kc][:, ga * NP_:(ga + 1) * NP_],
                                f2_t[kc][:, gb * NP_:(gb + 1) * NP_])
                            halves = [pt[:], pt[:]]
                        entry_tiles.append((entry, p3, halves))

                    for entry, p3, halves in entry_tiles:
                        for (l, p_idx, mlo, mhi, hlo, cgv) in entry["merges"]:
                            dm = mhi - mlo
                            dst = tp3[(l, p_idx)][:, mlo:mhi + 1:dm, :]
                            src = p3[:, 0:2, :] if hlo == 0 else p3[:, 1::-1, :]
                            klo, khi = (l, p_idx, mlo), (l, p_idx, mhi)
                            if klo not in written and khi not in written:
                                written.add(klo)
                                written.add(khi)
                                if USE_ACT_FIRST:
                                    nc.scalar.activation(
                                        dst, src, mybir.ActivationFunctionType.Copy,
                                        scale=cgv)
                                else:
                                    nc.vector.tensor_scalar(
                                        dst, src, cgv, None, mybir.AluOpType.mult)
                            elif klo in written and khi in written:
                                if merge_engine() is nc.gpsimd:
                                    tmp = prod2_pool.tile([P, 2 * NP_], DT, tag="tmp2",
                                                          name=f"tm_{kc}_{l}_{p_idx}_{mlo}_{mhi}")
                                    t3 = tmp[:].rearrange("p (g n) -> p g n", n=NP_)
                                    nc.vector.tensor_scalar(
                                        t3[:, 0:2, :], src, cgv, None,
                                        mybir.AluOpType.mult)
                                    nc.gpsimd.tensor_add(dst, t3[:, 0:2, :], dst)
                                else:
                                    nc.vector.scalar_tensor_tensor(
                                        dst, src, cgv, dst,
                                        mybir.AluOpType.mult, mybir.AluOpType.add)
                            else:
                                for key, m3, h in ((klo, mlo, hlo), (khi, mhi, 1 - hlo)):
                                    d1 = tp3[(l, p_idx)][:, m3:m3 + 1, :]
                                    if key not in written:
                                        written.add(key)
                                        nc.vector.tensor_scalar(
                                            d1, p3[:, h:h + 1, :], cgv, None,
                                            mybir.AluOpType.mult)
                                    else:
                                        slice_engine(l, p_idx, m3).scalar_tensor_tensor(
                                            d1, p3[:, h:h + 1, :], cgv, d1,
                                            mybir.AluOpType.mult, mybir.AluOpType.add)

                        for (l, p_idx, m3, h, cgv) in entry["singles"]:
                            key = (l, p_idx, m3)
                            dst = tp3[(l, p_idx)][:, m3:m3 + 1, :]
                            src1 = halves[h].rearrange("p (g n) -> p g n", g=1)
                            if key not in written:
                                written.add(key)
                                if USE_ACT_FIRST:
                                    nc.scalar.activation(
                                        dst, src1,
                                        mybir.ActivationFunctionType.Copy, scale=cgv)
                                else:
                                    nc.vector.tensor_scalar(
                                        dst, src1, cgv, None, mybir.AluOpType.mult)
                            else:
                                slice_engine(l, p_idx, m3).scalar_tensor_tensor(
                                    dst, src1, cgv, dst,
                                    mybir.AluOpType.mult, mybir.AluOpType.add)

                    for l, ready in L_READY_AT.items():
                        if ci == ready:
                            emit_matmuls(l, kc, tp_t)
                flush_evicts()
            loop_ctx.close()


    nc.finalize()
    return nc


def _get_nc():
    global _BUILT
    if _BUILT is None:
        _BUILT = _build()
    return _BUILT


def _prep_in_maps(inputs):
    f1 = [np.asarray(inputs[f"f1_l{l}"], dtype=np.float32) for l in range(4)]
    U = [np.asarray(inputs[f"U_{l}"], dtype=np.float32) for l in range(4)]
    W = [np.asarray(inputs[f"W_{l}"], dtype=np.float32) for l in range(4)]
    up = np.concatenate(
        [(U[l].astype(np.float64) @ W[l].astype(np.float64)) for l in range(4)],
        axis=0)
    up_dev = np.ascontiguousarray(up.reshape(-1, P, K)).astype(NP_DT)

    f1_all = np.concatenate(f1, axis=1)
    f2_all = np.concatenate(
        [np.asarray(inputs[f"f2_l{l}"], dtype=np.float32) for l in range(4)], axis=1)

    in_maps = []
    for c in range(NCORES):
        sl = slice(c * NP_, (c + 1) * NP_)
        f1t = np.ascontiguousarray(f1_all[sl].transpose(2, 1, 0)).reshape(
            NKC, P, A_TOT * NP_).astype(NP_DT)
        f2t = np.ascontiguousarray(f2_all[sl].transpose(2, 1, 0)).reshape(
            NKC, P, A_TOT * NP_).astype(NP_DT)
        in_maps.append({"f1t": f1t, "f2t": f2t, "up": up_dev})
    return f1, in_maps


def kernel(**inputs):
    f1, in_maps = _prep_in_maps(inputs)
    res = run_bass_kernel_spmd(_get_nc(), in_maps, list(range(NCORES)))

    outs = []
    for l in range(4):
        pieces = []
        for c in range(NCORES):
            part = res.results[c][f"outT{l}"]
            mixed = (part[0] + part[1]).reshape(K, 2 * l + 1, NP_)
            pieces.append(mixed.transpose(2, 1, 0))
        mixed_full = np.concatenate(pieces, axis=0)
        outs.append((f1[l] + mixed_full).astype(np.float32))
    return tuple(outs)


# revision 26
# speedup vs baseline: 1.0008x; 1.0008x over previous
"""Trainium2 Bass kernel for the CG tensor-product iteration (nn_CGIteration).

Computation per l (L_MAX=3, K=256, N=2048):
    tp_l   = concat_{(l1,l2) in PATHS[l]} einsum('abm,nak,nbk->nmk', CG, f1_l1, f2_l2)
    out_l  = f1_l + (tp_l @ U_l) @ W_l

Sharding: data-parallel over N across 8 cores (N/8 = 256 per core).

Device-side layout: channels-on-partitions.  Host pre-transposes f1/f2 to
[k, a, n] per core, precomputes U'_l = U_l @ W_l (fp64) and the residual add +
final transpose happen on host.  On-chip per core:
  1. products P_ab[k, n] = f1T[:, a, :] * f2T[:, b, :]          (DVE/GpSimd, fp16)
  2. CG FMAs  tpT[(p,k), (m, n)] += cg * P_ab                   (DVE + ACT, fp16)
     - first write to a tp slice is a scaled copy on the Scalar engine
     - (m3, -m3) partner terms of even paths merge into one 2-point op
  3. U' matmul: psum[j, (m,n)] += U'[(p,k), j].T @ tpT           (PE, fp16 -> fp32)
     The two k-half phases are written to DRAM as separate fp32 partials
     and summed on the host together with the residual.
"""

import os
from contextlib import ExitStack
from math import factorial

import numpy as np

import concourse.bass as bass
import concourse.mybir as mybir
from concourse import bacc, tile
from concourse.bass_utils import run_bass_kernel_spmd

L_MAX = 3
K = 256
N = 2048
NCORES = 8
NP_ = N // NCORES          # samples per core (256)
P = 128                    # SBUF partitions
NKC = K // P               # k-half chunks (2)
A_OFF = [0, 1, 4, 9]       # offset of each l-block among the 16 m-components
A_TOT = 16

PATHS = {l: [(l1, l2) for l1 in range(L_MAX + 1) for l2 in range(L_MAX + 1)
             if abs(l1 - l2) <= l <= l1 + l2] for l in range(L_MAX + 1)}
CIN = {l: len(PATHS[l]) * K for l in range(L_MAX + 1)}

# fraction of ops routed to GpSimd (DVE is the bottleneck engine)
GPS_PROD_FRAC = float(os.environ.get("GPS_PROD_FRAC", "0.55"))
GPS_SINGLE_FRAC = float(os.environ.get("GPS_SINGLE_FRAC", "0.85"))
GPS_MERGE_FRAC = float(os.environ.get("GPS_MERGE_FRAC", "0.0"))
USE_PAIR = os.environ.get("PAIR", "1") == "1"
USE_ACT_FIRST = os.environ.get("ACT_FIRST", "1") == "1"
USE_PAIR_PROD = os.environ.get("PAIR_PROD", "1") == "1"
KERNEL_LOOP = int(os.environ.get("KERNEL_LOOP", "1"))


def _cg_coeff(l1, m1, l2, m2, l3, m3):
    if m3 != m1 + m2:
        return 0.0
    pref = ((2 * l3 + 1) * factorial(l3 + l1 - l2) * factorial(l3 - l1 + l2)
            * factorial(l1 + l2 - l3) / factorial(l1 + l2 + l3 + 1)) ** 0.5
    pref *= (factorial(l3 + m3) * factorial(l3 - m3) * factorial(l1 - m1)
             * factorial(l1 + m1) * factorial(l2 - m2) * factorial(l2 + m2)) ** 0.5
    s = 0.0
    for k in range(0, l1 + l2 - l3 + 1):
        d = [k, l1 + l2 - l3 - k, l1 - m1 - k, l2 + m2 - k,
             l3 - l2 + m1 + k, l3 - l1 - m2 + k]
        if min(d) < 0:
            continue
        den = 1.0
        for x in d:
            den *= factorial(x)
        s += (-1.0) ** k / den
    return pref * s


def _cg_tensor(l1, l2, l3):
    out = np.zeros((2 * l1 + 1, 2 * l2 + 1, 2 * l3 + 1), dtype=np.float64)
    for m1 in range(-l1, l1 + 1):
        for m2 in range(-l2, l2 + 1):
            m3 = m1 + m2
            if -l3 <= m3 <= l3:
                out[m1 + l1, m2 + l2, m3 + l3] = _cg_coeff(l1, m1, l2, m2, l3, m3)
    return out


CG = {(l1, l2, l): _cg_tensor(l1, l2, l)
      for l in range(L_MAX + 1) for (l1, l2) in PATHS[l]}

# combo order: complete the heaviest output l first so its matmuls overlap
# with the remaining CG work
COMBO_ORDER = ([(0, 3), (1, 2), (1, 3), (2, 1), (2, 2), (2, 3), (3, 0), (3, 1),
                (3, 2), (3, 3)] + [(0, 2), (1, 1), (2, 0)] + [(0, 1), (1, 0)]
               + [(0, 0)])
# after which combo (index) each l's paths are all complete
L_READY_AT = {3: 9, 2: 12, 1: 14, 0: 15}

# ---------------------------------------------------------------------------
# Static op schedule.  Per combo: a list of "orbit" entries.  Each entry:
#   prods: [(ga, gb)] or [(ga, gb), (gpa, gpb)]   global m-component indices
#   merges: [(l, p_idx, mlo, mhi, half_of_mlo, cg)]  2-point ops (mhi > mlo)
#   singles: [(l, p_idx, m3, half, cg)]
# ---------------------------------------------------------------------------


def _build_schedule():
    sched = {}
    for (l1, l2) in COMBO_ORDER:
        # gather all nonzero CG terms of this combo: (a, b) -> [(l, p, m3, cg)]
        uses = {}
        for l in range(L_MAX + 1):
            if (l1, l2) not in PATHS[l]:
                continue
            p_idx = PATHS[l].index((l1, l2))
            cg = CG[(l1, l2, l)]
            for a in range(2 * l1 + 1):
                for b in range(2 * l2 + 1):
                    for m3 in range(2 * l + 1):
                        c = cg[a, b, m3]
                        if abs(c) > 1e-12:
                            uses.setdefault((a, b), []).append((l, p_idx, m3, float(c)))

        entries = []
        done_ab = set()
        for (a, b) in sorted(uses.keys()):
            if (a, b) in done_ab:
                continue
            pa, pb = 2 * l1 - a, 2 * l2 - b
            self_paired = (pa, pb) == (a, b)
            if self_paired or (pa, pb) not in uses or not USE_PAIR:
                done_ab.add((a, b))
                prods = [(A_OFF[l1] + a, A_OFF[l2] + b)]
                singles = [(l, p, m3, 0, c) for (l, p, m3, c) in uses[(a, b)]]
                entries.append(dict(prods=prods, merges=[], singles=singles))
                continue
            # two-product orbit
            done_ab.add((a, b))
            done_ab.add((pa, pb))
            prods = [(A_OFF[l1] + a, A_OFF[l2] + b),
                     (A_OFF[l1] + pa, A_OFF[l2] + pb)]
            merges, singles = [], []
            emitted = set()
            for half, (aa, bb) in enumerate([(a, b), (pa, pb)]):
                for (l, p, m3, c) in uses[(aa, bb)]:
                    if (l, p, m3, half) in emitted:
                        continue
                    s = (-1) ** (l1 + l2 - l)
                    pm = 2 * l - m3
                    if s == 1 and pm != m3:
                        # partner term lives on the other half with same cg
                        oh = 1 - half
                        if m3 < pm:
                            merges.append((l, p, m3, pm, half, c))
                        else:
                            merges.append((l, p, pm, m3, oh, c))
                        emitted.add((l, p, m3, half))
                        emitted.add((l, p, pm, oh))
                    elif (l, p, m3, half) not in emitted:
                        singles.append((l, p, m3, half, c))
                        emitted.add((l, p, m3, half))
            # dedupe merges (each pair appears from both halves)
            merges = sorted(set(merges))
            entries.append(dict(prods=prods, merges=merges, singles=singles))
        sched[(l1, l2)] = entries
    return sched


SCHED = _build_schedule()

DT = mybir.dt.float16
NP_DT = np.float16

_BUILT = None


def _build():
    """Build the single-core SPMD Bass program (same on all 8 cores)."""
    nc = bacc.Bacc(None, target_bir_lowering=False)

    f1t_ext = nc.declare_dram_parameter("f1t", [NKC, P, A_TOT * NP_], DT, isOutput=False)
    f2t_ext = nc.declare_dram_parameter("f2t", [NKC, P, A_TOT * NP_], DT, isOutput=False)
    n_up_chunks = sum(CIN[l] for l in range(4)) // P       # 68
    up_ext = nc.declare_dram_parameter("up", [n_up_chunks, P, K], DT, isOutput=False)
    out_ext = [nc.declare_dram_parameter(f"outT{l}", [NKC, K, (2 * l + 1) * NP_],
                                         mybir.dt.float32, isOutput=True)
               for l in range(4)]

    up_chunk_base = {}
    acc_chunks = 0
    for l in range(4):
        up_chunk_base[l] = acc_chunks
        acc_chunks += CIN[l] // P

    # engine pickers: per-slice deterministic so an accumulation chain for a
    # given tp slice stays on one engine (cross-engine RAW hops stall the
    # in-order queues)
    def make_picker(frac):
        cnt = [0.0]

        def pick():
            cnt[0] += frac
            if cnt[0] >= 1.0:
                cnt[0] -= 1.0
                return nc.gpsimd
            return nc.vector
        return pick

    prod_engine = make_picker(GPS_PROD_FRAC)

    def slice_engine(l, p_idx, m3):
        h = (l * 131 + p_idx * 31 + m3 * 7) % 100
        return nc.gpsimd if h < GPS_SINGLE_FRAC * 100 else nc.vector

    merge_engine = make_picker(GPS_MERGE_FRAC)

    with tile.TileContext(nc) as tc:
        with (
            tc.tile_pool(name="inp", bufs=1) as inp_pool,
            tc.tile_pool(name="upp", bufs=1) as up_pool,
            tc.tile_pool(name="tp", bufs=1) as tp_pool,
            tc.tile_pool(name="prod2", bufs=24) as prod2_pool,
            tc.tile_pool(name="prod1", bufs=8) as prod1_pool,
            tc.tile_pool(name="stage", bufs=8) as stage_pool,
            tc.tile_pool(name="ps", bufs=8, space="PSUM") as ps_pool,
        ):
            loop_ctx = ExitStack()
            if KERNEL_LOOP > 1:
                loop_ctx.enter_context(tc.For_i(0, KERNEL_LOOP, 1))

            up_t = up_pool.tile([P, n_up_chunks * K], DT, tag="up")
            nc.sync.dma_start(out=up_t[:], in_=up_ext.rearrange("c p k -> p c k"))

            f1_t = [inp_pool.tile([P, A_TOT * NP_], DT, tag=f"f1_{kc}", name=f"f1_{kc}")
                    for kc in range(NKC)]
            f2_t = [inp_pool.tile([P, A_TOT * NP_], DT, tag=f"f2_{kc}", name=f"f2_{kc}")
                    for kc in range(NKC)]
            for kc in range(NKC):
                nc.sync.dma_start(out=f1_t[kc][:], in_=f1t_ext[kc])
                nc.sync.dma_start(out=f2_t[kc][:], in_=f2t_ext[kc])

            RT = {l: [(i * 512, min(512, (2 * l + 1) * NP_ - i * 512))
                      for i in range(((2 * l + 1) * NP_ + 511) // 512)]
                  for l in range(4)}

            pending_evicts = []

            def emit_matmuls(l, kc, tp_t):
                flush_evicts()
                npaths = len(PATHS[l])
                for jh in range(2):
                    for ri, (r0, rw) in enumerate(RT[l]):
                        ps = ps_pool.tile([P, rw], mybir.dt.float32, tag="ps",
                                          name=f"ps_{l}_{jh}_{ri}_{kc}")
                        for pc in range(npaths):
                            chunk = up_chunk_base[l] + pc * NKC + kc
                            lhsT = up_t[:, chunk * K + jh * P: chunk * K + jh * P + P]
                            rhs = tp_t[(l, pc)][:, r0:r0 + rw]
                            nc.tensor.matmul(ps[:], lhsT, rhs,
                                             start=(pc == 0), stop=(pc == npaths - 1))
                        pending_evicts.append((ps, l, jh, ri, r0, rw, kc))

            def flush_evicts():
                for ps, l, jh, ri, r0, rw, kc in pending_evicts:
                    st = stage_pool.tile([P, rw], mybir.dt.float32, tag="stage",
                                         name=f"st_{l}_{jh}_{ri}_{kc}")
                    nc.scalar.activation(
                        st[:], ps[:], mybir.ActivationFunctionType.Copy)
                    nc.sync.dma_start(
                        out=out_ext[l][kc, jh * P:(jh + 1) * P, r0:r0 + rw],
                        in_=st[:])
                pending_evicts.clear()

            for kc in range(NKC):
                tp_t = {}
                tp3 = {}
                written = set()
                for l in range(4):
                    for p_idx in range(len(PATHS[l])):
                        t = tp_pool.tile([P, (2 * l + 1) * NP_], DT,
                                         tag=f"tp_{l}_{p_idx}", name=f"tp_{l}_{p_idx}_{kc}")
                        tp_t[(l, p_idx)] = t
                        tp3[(l, p_idx)] = t[:].rearrange("p (m n) -> p m n", n=NP_)

                for ci, (l1, l2) in enumerate(COMBO_ORDER):
                    entry_tiles = []
                    for entry in SCHED[(l1, l2)]:
                        prods = entry["prods"]
                        if len(prods) == 2:
                            (ga, gb), (gpa, gpb) = prods
                            da, db = gpa - ga, gpb - gb
                            pt = prod2_pool.tile([P, 2 * NP_], DT, tag="prod2",
                                                name=f"pp_{kc}_{l1}{l2}_{ga}_{gb}")
                            p3 = pt[:].rearrange("p (g n) -> p g n", n=NP_)
                            if USE_PAIR_PROD and da != 0 and db != 0:
                                # one 2-point-AP multiply computes both halves
                                b1 = f1_t[kc][:]
                                b2 = f2_t[kc][:]
                                in0 = bass.AP(b1.tensor, b1.offset + ga * NP_,
                                              [list(b1.ap[0]), [da * NP_, 2], [1, NP_]])
                                in1 = bass.AP(b2.tensor, b2.offset + gb * NP_,
                                              [list(b2.ap[0]), [db * NP_, 2], [1, NP_]])
                                prod_engine().tensor_mul(p3[:, 0:2, :], in0, in1)
                            else:
                                for h, (xa, xb) in enumerate(prods):
                                    prod_engine().tensor_mul(
                                        pt[:, h * NP_:(h + 1) * NP_],
                                        f1_t[kc][:, xa * NP_:(xa + 1) * NP_],
                                        f2_t[kc][:, xb * NP_:(xb + 1) * NP_])
                            halves = [pt[:, 0:NP_], pt[:, NP_:2 * NP_]]
                        else:
                            ga, gb = prods[0]
                            pt = prod1_pool.tile([P, NP_], DT, tag="prod1",
                                                name=f"pp_{kc}_{l1}{l2}_{ga}_{gb}")
                            p3 = None
                            prod_engine().tensor_mul(
                                pt[:],
                                f1_t[kc][:, ga * NP_:(ga + 1) * NP_],
                                f2_t[kc][:, gb * NP_:(gb + 1) * NP_])
                            halves = [pt[:], pt[:]]
                        entry_tiles.append((entry, p3, halves))

                    for entry, p3, halves in entry_tiles:
                        for (l, p_idx, mlo, mhi, hlo, cgv) in entry["merges"]:
                            dm = mhi - mlo
                            dst = tp3[(l, p_idx)][:, mlo:mhi + 1:dm, :]
                            src = p3[:, 0:2, :] if hlo == 0 else p3[:, 1::-1, :]
                            klo, khi = (l, p_idx, mlo), (l, p_idx, mhi)
                            if klo not in written and khi not in written:
                                written.add(klo)
                                written.add(khi)
                                if USE_ACT_FIRST:
                                    nc.scalar.activation(
                                        dst, src, mybir.ActivationFunctionType.Copy,
                                        scale=cgv)
                                else:
                                    nc.vector.tensor_scalar(
                                        dst, src, cgv, None, mybir.AluOpType.mult)
                            elif klo in written and khi in written:
                                if merge_engine() is nc.gpsimd:
                                    tmp = prod2_pool.tile([P, 2 * NP_], DT, tag="tmp2",
                                                          name=f"tm_{kc}_{l}_{p_idx}_{mlo}_{mhi}")
                                    t3 = tmp[:].rearrange("p (g n) -> p g n", n=NP_)
                                    nc.vector.tensor_scalar(
                                        t3[:, 0:2, :], src, cgv, None,
                                        mybir.AluOpType.mult)
                                    nc.gpsimd.tensor_add(dst, t3[:, 0:2, :], dst)
                                else:
                                    nc.vector.scalar_tensor_tensor(
                                        dst, src, cgv, dst,
                                        mybir.AluOpType.mult, mybir.AluOpType.add)
                            else:
                                for key, m3, h in ((klo, mlo, hlo), (khi, mhi, 1 - hlo)):
                                    d1 = tp3[(l, p_idx)][:, m3:m3 + 1, :]
                                    if key not in written:
                                        written.add(key)
                                        nc.vector.tensor_scalar(
                                            d1, p3[:, h:h + 1, :], cgv, None,
                                            mybir.AluOpType.mult)
                                    else:
                                        slice_engine(l, p_idx, m3).scalar_tensor_tensor(
                                            d1, p3[:, h:h + 1, :], cgv, d1,
                                            mybir.AluOpType.mult, mybir.AluOpType.add)

                        for (l, p_idx, m3, h, cgv) in entry["singles"]:
                            key = (l, p_idx, m3)
                            dst = tp3[(l, p_idx)][:, m3:m3 + 1, :]
                            src1 = halves[h].rearrange("p (g n) -> p g n", g=1)
                            if key not in written:
                                written.add(key)
                                if USE_ACT_FIRST:
                                    nc.scalar.activation(
                                        dst, src1,
                                        mybir.ActivationFunctionType.Copy, scale=cgv)
                                else:
                                    nc.vector.tensor_scalar(
                                        dst, src1, cgv, None, mybir.AluOpType.mult)
                            else:
                                slice_engine(l, p_idx, m3).scalar_tensor_tensor(
                                    dst, src1, cgv, dst,
                                    mybir.AluOpType.mult, mybir.AluOpType.add)

                    for l, ready in L_READY_AT.items():
                        if ci == ready:
                            emit_matmuls(l, kc, tp_t)
                flush_evicts()
            loop_ctx.close()


    nc.finalize()
    return nc


def _get_nc():
    global _BUILT
    if _BUILT is None:
        _BUILT = _build()
    return _BUILT


def _prep_in_maps(inputs):
    f1 = [np.asarray(inputs[f"f1_l{l}"], dtype=np.float32) for l in range(4)]
    U = [np.asarray(inputs[f"U_{l}"], dtype=np.float32) for l in range(4)]
    W = [np.asarray(inputs[f"W_{l}"], dtype=np.float32) for l in range(4)]
    up = np.concatenate(
        [(U[l].astype(np.float64) @ W[l].astype(np.float64)) for l in range(4)],
        axis=0)
    up_dev = np.ascontiguousarray(up.reshape(-1, P, K)).astype(NP_DT)

    f1_all = np.concatenate(f1, axis=1)
    f2_all = np.concatenate(
        [np.asarray(inputs[f"f2_l{l}"], dtype=np.float32) for l in range(4)], axis=1)

    in_maps = []
    for c in range(NCORES):
        sl = slice(c * NP_, (c + 1) * NP_)
        f1t = np.ascontiguousarray(f1_all[sl].transpose(2, 1, 0)).reshape(
            NKC, P, A_TOT * NP_).astype(NP_DT)
        f2t = np.ascontiguousarray(f2_all[sl].transpose(2, 1, 0)).reshape(
            NKC, P, A_TOT * NP_).astype(NP_DT)
        in_maps.append({"f1t": f1t, "f2t": f2t, "up": up_dev})
    return f1, in_maps


def kernel(**inputs):
    f1, in_maps = _prep_in_maps(inputs)
    res = run_bass_kernel_spmd(_get_nc(), in_maps, list(range(NCORES)))

    outs = []
    for l in range(4):
        pieces = []
        for c in range(NCORES):
            part = res.results[c][f"outT{l}"]
            mixed = (part[0] + part[1]).reshape(K, 2 * l + 1, NP_)
            pieces.append(mixed.transpose(2, 1, 0))
        mixed_full = np.concatenate(pieces, axis=0)
        outs.append((f1[l] + mixed_full).astype(np.float32))
    return tuple(outs)


# revision 30
# speedup vs baseline: 1.0110x; 1.0102x over previous
"""Trainium2 Bass kernel for the CG tensor-product iteration (nn_CGIteration).

Computation per l (L_MAX=3, K=256, N=2048):
    tp_l   = concat_{(l1,l2) in PATHS[l]} einsum('abm,nak,nbk->nmk', CG, f1_l1, f2_l2)
    out_l  = f1_l + (tp_l @ U_l) @ W_l

Sharding: data-parallel over N across 8 cores (N/8 = 256 per core).

Device-side layout: channels-on-partitions.  Host pre-transposes f1/f2 to
[k, a, n] per core, precomputes U'_l = U_l @ W_l (fp64) and the residual add +
final transpose happen on host.  On-chip per core:
  1. products P_ab[k, n] = f1T[:, a, :] * f2T[:, b, :]          (DVE/GpSimd, fp16)
  2. CG FMAs  tpT[(p,k), (m, n)] += cg * P_ab                   (DVE + ACT, fp16)
     - first write to a tp slice is a scaled copy on the Scalar engine
     - (m3, -m3) partner terms of even paths merge into one 2-point op
  3. U' matmul: psum[j, (m,n)] += U'[(p,k), j].T @ tpT           (PE, fp16 -> fp32)
     The two k-half phases are written to DRAM as separate fp32 partials
     and summed on the host together with the residual.
"""

import os
from contextlib import ExitStack
from math import factorial

import numpy as np

import concourse.bass as bass
import concourse.mybir as mybir
from concourse import bacc, tile
from concourse.bass_utils import run_bass_kernel_spmd

L_MAX = 3
K = 256
N = 2048
NCORES = 8
NP_ = N // NCORES          # samples per core (256)
P = 128                    # SBUF partitions
NKC = K // P               # k-half chunks (2)
A_OFF = [0, 1, 4, 9]       # offset of each l-block among the 16 m-components
A_TOT = 16

PATHS = {l: [(l1, l2) for l1 in range(L_MAX + 1) for l2 in range(L_MAX + 1)
             if abs(l1 - l2) <= l <= l1 + l2] for l in range(L_MAX + 1)}
CIN = {l: len(PATHS[l]) * K for l in range(L_MAX + 1)}

# fraction of ops routed to GpSimd (DVE is the bottleneck engine)
GPS_PROD_FRAC = float(os.environ.get("GPS_PROD_FRAC", "0.55"))
GPS_SINGLE_FRAC = float(os.environ.get("GPS_SINGLE_FRAC", "0.85"))
GPS_MERGE_FRAC = float(os.environ.get("GPS_MERGE_FRAC", "0.0"))
USE_PAIR = os.environ.get("PAIR", "1") == "1"
USE_ACT_FIRST = os.environ.get("ACT_FIRST", "1") == "1"
USE_PAIR_PROD = os.environ.get("PAIR_PROD", "1") == "1"
KERNEL_LOOP = int(os.environ.get("KERNEL_LOOP", "1"))


def _cg_coeff(l1, m1, l2, m2, l3, m3):
    if m3 != m1 + m2:
        return 0.0
    pref = ((2 * l3 + 1) * factorial(l3 + l1 - l2) * factorial(l3 - l1 + l2)
            * factorial(l1 + l2 - l3) / factorial(l1 + l2 + l3 + 1)) ** 0.5
    pref *= (factorial(l3 + m3) * factorial(l3 - m3) * factorial(l1 - m1)
             * factorial(l1 + m1) * factorial(l2 - m2) * factorial(l2 + m2)) ** 0.5
    s = 0.0
    for k in range(0, l1 + l2 - l3 + 1):
        d = [k, l1 + l2 - l3 - k, l1 - m1 - k, l2 + m2 - k,
             l3 - l2 + m1 + k, l3 - l1 - m2 + k]
        if min(d) < 0:
            continue
        den = 1.0
        for x in d:
            den *= factorial(x)
        s += (-1.0) ** k / den
    return pref * s


def _cg_tensor(l1, l2, l3):
    out = np.zeros((2 * l1 + 1, 2 * l2 + 1, 2 * l3 + 1), dtype=np.float64)
    for m1 in range(-l1, l1 + 1):
        for m2 in range(-l2, l2 + 1):
            m3 = m1 + m2
            if -l3 <= m3 <= l3:
                out[m1 + l1, m2 + l2, m3 + l3] = _cg_coeff(l1, m1, l2, m2, l3, m3)
    return out


CG = {(l1, l2, l): _cg_tensor(l1, l2, l)
      for l in range(L_MAX + 1) for (l1, l2) in PATHS[l]}

# combo order: complete the heaviest output l first so its matmuls overlap
# with the remaining CG work
COMBO_ORDER = ([(0, 3), (1, 2), (1, 3), (2, 1), (2, 2), (2, 3), (3, 0), (3, 1),
                (3, 2), (3, 3)] + [(0, 2), (1, 1), (2, 0)] + [(0, 1), (1, 0)]
               + [(0, 0)])
# after which combo (index) each l's paths are all complete
L_READY_AT = {3: 9, 2: 12, 1: 14, 0: 15}

# ---------------------------------------------------------------------------
# Static op schedule.  Per combo: a list of "orbit" entries.  Each entry:
#   prods: [(ga, gb)] or [(ga, gb), (gpa, gpb)]   global m-component indices
#   merges: [(l, p_idx, mlo, mhi, half_of_mlo, cg)]  2-point ops (mhi > mlo)
#   singles: [(l, p_idx, m3, half, cg)]
# ---------------------------------------------------------------------------


def _build_schedule():
    sched = {}
    for (l1, l2) in COMBO_ORDER:
        # gather all nonzero CG terms of this combo: (a, b) -> [(l, p, m3, cg)]
        uses = {}
        for l in range(L_MAX + 1):
            if (l1, l2) not in PATHS[l]:
                continue
            p_idx = PATHS[l].index((l1, l2))
            cg = CG[(l1, l2, l)]
            for a in range(2 * l1 + 1):
                for b in range(2 * l2 + 1):
                    for m3 in range(2 * l + 1):
                        c = cg[a, b, m3]
                        if abs(c) > 1e-12:
                            uses.setdefault((a, b), []).append((l, p_idx, m3, float(c)))

        entries = []
        done_ab = set()
        for (a, b) in sorted(uses.keys()):
            if (a, b) in done_ab:
                continue
            pa, pb = 2 * l1 - a, 2 * l2 - b
            self_paired = (pa, pb) == (a, b)
            if self_paired or (pa, pb) not in uses or not USE_PAIR:
                done_ab.add((a, b))
                prods = [(A_OFF[l1] + a, A_OFF[l2] + b)]
                singles = [(l, p, m3, 0, c) for (l, p, m3, c) in uses[(a, b)]]
                entries.append(dict(prods=prods, merges=[], singles=singles))
                continue
            # two-product orbit
            done_ab.add((a, b))
            done_ab.add((pa, pb))
            prods = [(A_OFF[l1] + a, A_OFF[l2] + b),
                     (A_OFF[l1] + pa, A_OFF[l2] + pb)]
            merges, singles = [], []
            emitted = set()
            for half, (aa, bb) in enumerate([(a, b), (pa, pb)]):
                for (l, p, m3, c) in uses[(aa, bb)]:
                    if (l, p, m3, half) in emitted:
                        continue
                    s = (-1) ** (l1 + l2 - l)
                    pm = 2 * l - m3
                    if s == 1 and pm != m3:
                        # partner term lives on the other half with same cg
                        oh = 1 - half
                        if m3 < pm:
                            merges.append((l, p, m3, pm, half, c))
                        else:
                            merges.append((l, p, pm, m3, oh, c))
                        emitted.add((l, p, m3, half))
                        emitted.add((l, p, pm, oh))
                    elif (l, p, m3, half) not in emitted:
                        singles.append((l, p, m3, half, c))
                        emitted.add((l, p, m3, half))
            # dedupe merges (each pair appears from both halves)
            merges = sorted(set(merges))
            entries.append(dict(prods=prods, merges=merges, singles=singles))
        sched[(l1, l2)] = entries
    return sched


SCHED = _build_schedule()

DT = mybir.dt.float16
NP_DT = np.float16

_BUILT = None


def _build():
    """Build the single-core SPMD Bass program (same on all 8 cores)."""
    nc = bacc.Bacc(None, target_bir_lowering=False)

    f1t_ext = nc.declare_dram_parameter("f1t", [NKC, P, A_TOT * NP_], DT, isOutput=False)
    f2t_ext = nc.declare_dram_parameter("f2t", [NKC, P, A_TOT * NP_], DT, isOutput=False)
    n_up_chunks = sum(CIN[l] for l in range(4)) // P       # 68
    up_ext = nc.declare_dram_parameter("up", [n_up_chunks, P, K], DT, isOutput=False)
    out_ext = [nc.declare_dram_parameter(f"outT{l}", [NKC, K, (2 * l + 1) * NP_],
                                         mybir.dt.float32, isOutput=True)
               for l in range(4)]

    up_chunk_base = {}
    acc_chunks = 0
    for l in range(4):
        up_chunk_base[l] = acc_chunks
        acc_chunks += CIN[l] // P

    # engine pickers: per-slice deterministic so an accumulation chain for a
    # given tp slice stays on one engine (cross-engine RAW hops stall the
    # in-order queues)
    def make_picker(frac):
        cnt = [0.0]

        def pick():
            cnt[0] += frac
            if cnt[0] >= 1.0:
                cnt[0] -= 1.0
                return nc.gpsimd
            return nc.vector
        return pick

    prod_engine = make_picker(GPS_PROD_FRAC)

    def slice_engine(l, p_idx, m3):
        h = (l * 131 + p_idx * 31 + m3 * 7) % 100
        return nc.gpsimd if h < GPS_SINGLE_FRAC * 100 else nc.vector

    merge_engine = make_picker(GPS_MERGE_FRAC)

    with tile.TileContext(nc) as tc:
        with (
            tc.tile_pool(name="inp", bufs=1) as inp_pool,
            tc.tile_pool(name="upp", bufs=1) as up_pool,
            tc.tile_pool(name="tp", bufs=1) as tp_pool,
            tc.tile_pool(name="prod2", bufs=40) as prod2_pool,
            tc.tile_pool(name="prod1", bufs=8) as prod1_pool,
            tc.tile_pool(name="stage", bufs=8) as stage_pool,
            tc.tile_pool(name="ps", bufs=8, space="PSUM") as ps_pool,
        ):
            loop_ctx = ExitStack()
            if KERNEL_LOOP > 1:
                loop_ctx.enter_context(tc.For_i(0, KERNEL_LOOP, 1))

            up_t = up_pool.tile([P, n_up_chunks * K], DT, tag="up")
            nc.sync.dma_start(out=up_t[:], in_=up_ext.rearrange("c p k -> p c k"))

            f1_t = [inp_pool.tile([P, A_TOT * NP_], DT, tag=f"f1_{kc}", name=f"f1_{kc}")
                    for kc in range(NKC)]
            f2_t = [inp_pool.tile([P, A_TOT * NP_], DT, tag=f"f2_{kc}", name=f"f2_{kc}")
                    for kc in range(NKC)]
            for kc in range(NKC):
                nc.sync.dma_start(out=f1_t[kc][:], in_=f1t_ext[kc])
                nc.sync.dma_start(out=f2_t[kc][:], in_=f2t_ext[kc])

            RT = {l: [(i * 512, min(512, (2 * l + 1) * NP_ - i * 512))
                      for i in range(((2 * l + 1) * NP_ + 511) // 512)]
                  for l in range(4)}

            pending_evicts = []

            def emit_matmuls(l, kc, tp_t):
                flush_evicts()
                npaths = len(PATHS[l])
                for jh in range(2):
                    for ri, (r0, rw) in enumerate(RT[l]):
                        ps = ps_pool.tile([P, rw], mybir.dt.float32, tag="ps",
                                          name=f"ps_{l}_{jh}_{ri}_{kc}")
                        for pc in range(npaths):
                            chunk = up_chunk_base[l] + pc * NKC + kc
                            lhsT = up_t[:, chunk * K + jh * P: chunk * K + jh * P + P]
                            rhs = tp_t[(l, pc)][:, r0:r0 + rw]
                            nc.tensor.matmul(ps[:], lhsT, rhs,
                                             start=(pc == 0), stop=(pc == npaths - 1))
                        pending_evicts.append((ps, l, jh, ri, r0, rw, kc))

            def flush_evicts():
                for ps, l, jh, ri, r0, rw, kc in pending_evicts:
                    st = stage_pool.tile([P, rw], mybir.dt.float32, tag="stage",
                                         name=f"st_{l}_{jh}_{ri}_{kc}")
                    nc.scalar.activation(
                        st[:], ps[:], mybir.ActivationFunctionType.Copy)
                    nc.sync.dma_start(
                        out=out_ext[l][kc, jh * P:(jh + 1) * P, r0:r0 + rw],
                        in_=st[:])
                pending_evicts.clear()

            for kc in range(NKC):
                tp_t = {}
                tp3 = {}
                written = set()
                for l in range(4):
                    for p_idx in range(len(PATHS[l])):
                        t = tp_pool.tile([P, (2 * l + 1) * NP_], DT,
                                         tag=f"tp_{l}_{p_idx}", name=f"tp_{l}_{p_idx}_{kc}")
                        tp_t[(l, p_idx)] = t
                        tp3[(l, p_idx)] = t[:].rearrange("p (m n) -> p m n", n=NP_)

                for ci, (l1, l2) in enumerate(COMBO_ORDER):
                    entry_tiles = []
                    for entry in SCHED[(l1, l2)]:
                        prods = entry["prods"]
                        if len(prods) == 2:
                            (ga, gb), (gpa, gpb) = prods
                            da, db = gpa - ga, gpb - gb
                            pt = prod2_pool.tile([P, 2 * NP_], DT, tag="prod2",
                                                name=f"pp_{kc}_{l1}{l2}_{ga}_{gb}")
                            p3 = pt[:].rearrange("p (g n) -> p g n", n=NP_)
                            if USE_PAIR_PROD and da != 0 and db != 0:
                                # one 2-point-AP multiply computes both halves
                                b1 = f1_t[kc][:]
                                b2 = f2_t[kc][:]
                                in0 = bass.AP(b1.tensor, b1.offset + ga * NP_,
                                              [list(b1.ap[0]), [da * NP_, 2], [1, NP_]])
                                in1 = bass.AP(b2.tensor, b2.offset + gb * NP_,
                                              [list(b2.ap[0]), [db * NP_, 2], [1, NP_]])
                                prod_engine().tensor_mul(p3[:, 0:2, :], in0, in1)
                            else:
                                for h, (xa, xb) in enumerate(prods):
                                    prod_engine().tensor_mul(
                                        pt[:, h * NP_:(h + 1) * NP_],
                                        f1_t[kc][:, xa * NP_:(xa + 1) * NP_],
                                        f2_t[kc][:, xb * NP_:(xb + 1) * NP_])
                            halves = [pt[:, 0:NP_], pt[:, NP_:2 * NP_]]
                        else:
                            ga, gb = prods[0]
                            pt = prod1_pool.tile([P, NP_], DT, tag="prod1",
                                                name=f"pp_{kc}_{l1}{l2}_{ga}_{gb}")
                            p3 = None
                            prod_engine().tensor_mul(
                                pt[:],
                                f1_t[kc][:, ga * NP_:(ga + 1) * NP_],
                                f2_t[kc][:, gb * NP_:(gb + 1) * NP_])
                            halves = [pt[:], pt[:]]
                        entry_tiles.append((entry, p3, halves))

                    for entry, p3, halves in entry_tiles:
                        for (l, p_idx, mlo, mhi, hlo, cgv) in entry["merges"]:
                            dm = mhi - mlo
                            dst = tp3[(l, p_idx)][:, mlo:mhi + 1:dm, :]
                            src = p3[:, 0:2, :] if hlo == 0 else p3[:, 1::-1, :]
                            klo, khi = (l, p_idx, mlo), (l, p_idx, mhi)
                            if klo not in written and khi not in written:
                                written.add(klo)
                                written.add(khi)
                                if USE_ACT_FIRST:
                                    nc.scalar.activation(
                                        dst, src, mybir.ActivationFunctionType.Copy,
                                        scale=cgv)
                                else:
                                    nc.vector.tensor_scalar(
                                        dst, src, cgv, None, mybir.AluOpType.mult)
                            elif klo in written and khi in written:
                                if merge_engine() is nc.gpsimd:
                                    tmp = prod2_pool.tile([P, 2 * NP_], DT, tag="tmp2",
                                                          name=f"tm_{kc}_{l}_{p_idx}_{mlo}_{mhi}")
                                    t3 = tmp[:].rearrange("p (g n) -> p g n", n=NP_)
                                    nc.vector.tensor_scalar(
                                        t3[:, 0:2, :], src, cgv, None,
                                        mybir.AluOpType.mult)
                                    nc.gpsimd.tensor_add(dst, t3[:, 0:2, :], dst)
                                else:
                                    nc.vector.scalar_tensor_tensor(
                                        dst, src, cgv, dst,
                                        mybir.AluOpType.mult, mybir.AluOpType.add)
                            else:
                                for key, m3, h in ((klo, mlo, hlo), (khi, mhi, 1 - hlo)):
                                    d1 = tp3[(l, p_idx)][:, m3:m3 + 1, :]
                                    if key not in written:
                                        written.add(key)
                                        nc.vector.tensor_scalar(
                                            d1, p3[:, h:h + 1, :], cgv, None,
                                            mybir.AluOpType.mult)
                                    else:
                                        slice_engine(l, p_idx, m3).scalar_tensor_tensor(
                                            d1, p3[:, h:h + 1, :], cgv, d1,
                                            mybir.AluOpType.mult, mybir.AluOpType.add)

                        for (l, p_idx, m3, h, cgv) in entry["singles"]:
                            key = (l, p_idx, m3)
                            dst = tp3[(l, p_idx)][:, m3:m3 + 1, :]
                            src1 = halves[h].rearrange("p (g n) -> p g n", g=1)
                            if key not in written:
                                written.add(key)
                                if USE_ACT_FIRST:
                                    nc.scalar.activation(
                                        dst, src1,
                                        mybir.ActivationFunctionType.Copy, scale=cgv)
                                else:
                                    nc.vector.tensor_scalar(
                                        dst, src1, cgv, None, mybir.AluOpType.mult)
                            else:
                                slice_engine(l, p_idx, m3).scalar_tensor_tensor(
                                    dst, src1, cgv, dst,
                                    mybir.AluOpType.mult, mybir.AluOpType.add)

                    for l, ready in L_READY_AT.items():
                        if ci == ready:
                            emit_matmuls(l, kc, tp_t)
                flush_evicts()
            loop_ctx.close()


    nc.finalize()
    return nc


def _get_nc():
    global _BUILT
    if _BUILT is None:
        _BUILT = _build()
    return _BUILT


def _prep_in_maps(inputs):
    f1 = [np.asarray(inputs[f"f1_l{l}"], dtype=np.float32) for l in range(4)]
    U = [np.asarray(inputs[f"U_{l}"], dtype=np.float32) for l in range(4)]
    W = [np.asarray(inputs[f"W_{l}"], dtype=np.float32) for l in range(4)]
    up = np.concatenate(
        [(U[l].astype(np.float64) @ W[l].astype(np.float64)) for l in range(4)],
        axis=0)
    up_dev = np.ascontiguousarray(up.reshape(-1, P, K)).astype(NP_DT)

    f1_all = np.concatenate(f1, axis=1)
    f2_all = np.concatenate(
        [np.asarray(inputs[f"f2_l{l}"], dtype=np.float32) for l in range(4)], axis=1)

    in_maps = []
    for c in range(NCORES):
        sl = slice(c * NP_, (c + 1) * NP_)
        f1t = np.ascontiguousarray(f1_all[sl].transpose(2, 1, 0)).reshape(
            NKC, P, A_TOT * NP_).astype(NP_DT)
        f2t = np.ascontiguousarray(f2_all[sl].transpose(2, 1, 0)).reshape(
            NKC, P, A_TOT * NP_).astype(NP_DT)
        in_maps.append({"f1t": f1t, "f2t": f2t, "up": up_dev})
    return f1, in_maps


def kernel(**inputs):
    f1, in_maps = _prep_in_maps(inputs)
    res = run_bass_kernel_spmd(_get_nc(), in_maps, list(range(NCORES)))

    outs = []
    for l in range(4):
        pieces = []
        for c in range(NCORES):
            part = res.results[c][f"outT{l}"]
            mixed = (part[0] + part[1]).reshape(K, 2 * l + 1, NP_)
            pieces.append(mixed.transpose(2, 1, 0))
        mixed_full = np.concatenate(pieces, axis=0)
        outs.append((f1[l] + mixed_full).astype(np.float32))
    return tuple(outs)


# revision 32
# speedup vs baseline: 1.0174x; 1.0063x over previous
"""Trainium2 Bass kernel for the CG tensor-product iteration (nn_CGIteration).

Computation per l (L_MAX=3, K=256, N=2048):
    tp_l   = concat_{(l1,l2) in PATHS[l]} einsum('abm,nak,nbk->nmk', CG, f1_l1, f2_l2)
    out_l  = f1_l + (tp_l @ U_l) @ W_l

Sharding: data-parallel over N across 8 cores (N/8 = 256 per core).

Device-side layout: channels-on-partitions.  Host pre-transposes f1/f2 to
[k, a, n] per core, precomputes U'_l = U_l @ W_l (fp64) and the residual add +
final transpose happen on host.  On-chip per core:
  1. products P_ab[k, n] = f1T[:, a, :] * f2T[:, b, :]          (DVE/GpSimd, fp16)
  2. CG FMAs  tpT[(p,k), (m, n)] += cg * P_ab                   (DVE + ACT, fp16)
     - first write to a tp slice is a scaled copy on the Scalar engine
     - (m3, -m3) partner terms of even paths merge into one 2-point op
  3. U' matmul: psum[j, (m,n)] += U'[(p,k), j].T @ tpT           (PE, fp16 -> fp32)
     The two k-half phases are written to DRAM as separate fp32 partials
     and summed on the host together with the residual.
"""

import os
from contextlib import ExitStack
from math import factorial

import numpy as np

import concourse.bass as bass
import concourse.mybir as mybir
from concourse import bacc, tile
from concourse.bass_utils import run_bass_kernel_spmd

L_MAX = 3
K = 256
N = 2048
NCORES = 8
NP_ = N // NCORES          # samples per core (256)
P = 128                    # SBUF partitions
NKC = K // P               # k-half chunks (2)
A_OFF = [0, 1, 4, 9]       # offset of each l-block among the 16 m-components
A_TOT = 16

PATHS = {l: [(l1, l2) for l1 in range(L_MAX + 1) for l2 in range(L_MAX + 1)
             if abs(l1 - l2) <= l <= l1 + l2] for l in range(L_MAX + 1)}
CIN = {l: len(PATHS[l]) * K for l in range(L_MAX + 1)}

# fraction of ops routed to GpSimd (DVE is the bottleneck engine)
GPS_PROD_FRAC = float(os.environ.get("GPS_PROD_FRAC", "0.55"))
GPS_SINGLE_FRAC = float(os.environ.get("GPS_SINGLE_FRAC", "0.85"))
GPS_MERGE_FRAC = float(os.environ.get("GPS_MERGE_FRAC", "0.0"))
USE_PAIR = os.environ.get("PAIR", "1") == "1"
USE_ACT_FIRST = os.environ.get("ACT_FIRST", "1") == "1"
USE_PAIR_PROD = os.environ.get("PAIR_PROD", "1") == "1"
KERNEL_LOOP = int(os.environ.get("KERNEL_LOOP", "1"))


def _cg_coeff(l1, m1, l2, m2, l3, m3):
    if m3 != m1 + m2:
        return 0.0
    pref = ((2 * l3 + 1) * factorial(l3 + l1 - l2) * factorial(l3 - l1 + l2)
            * factorial(l1 + l2 - l3) / factorial(l1 + l2 + l3 + 1)) ** 0.5
    pref *= (factorial(l3 + m3) * factorial(l3 - m3) * factorial(l1 - m1)
             * factorial(l1 + m1) * factorial(l2 - m2) * factorial(l2 + m2)) ** 0.5
    s = 0.0
    for k in range(0, l1 + l2 - l3 + 1):
        d = [k, l1 + l2 - l3 - k, l1 - m1 - k, l2 + m2 - k,
             l3 - l2 + m1 + k, l3 - l1 - m2 + k]
        if min(d) < 0:
            continue
        den = 1.0
        for x in d:
            den *= factorial(x)
        s += (-1.0) ** k / den
    return pref * s


def _cg_tensor(l1, l2, l3):
    out = np.zeros((2 * l1 + 1, 2 * l2 + 1, 2 * l3 + 1), dtype=np.float64)
    for m1 in range(-l1, l1 + 1):
        for m2 in range(-l2, l2 + 1):
            m3 = m1 + m2
            if -l3 <= m3 <= l3:
                out[m1 + l1, m2 + l2, m3 + l3] = _cg_coeff(l1, m1, l2, m2, l3, m3)
    return out


CG = {(l1, l2, l): _cg_tensor(l1, l2, l)
      for l in range(L_MAX + 1) for (l1, l2) in PATHS[l]}

# combo order: complete the heaviest output l first so its matmuls overlap
# with the remaining CG work
COMBO_ORDER = ([(0, 3), (1, 2), (1, 3), (2, 1), (2, 2), (2, 3), (3, 0), (3, 1),
                (3, 2), (3, 3)] + [(0, 2), (1, 1), (2, 0)] + [(0, 1), (1, 0)]
               + [(0, 0)])
# after which combo (index) each l's paths are all complete
L_READY_AT = {3: 9, 2: 12, 1: 14, 0: 15}

# ---------------------------------------------------------------------------
# Static op schedule.  Per combo: a list of "orbit" entries.  Each entry:
#   prods: [(ga, gb)] or [(ga, gb), (gpa, gpb)]   global m-component indices
#   merges: [(l, p_idx, mlo, mhi, half_of_mlo, cg)]  2-point ops (mhi > mlo)
#   singles: [(l, p_idx, m3, half, cg)]
# ---------------------------------------------------------------------------


def _build_schedule():
    sched = {}
    for (l1, l2) in COMBO_ORDER:
        # gather all nonzero CG terms of this combo: (a, b) -> [(l, p, m3, cg)]
        uses = {}
        for l in range(L_MAX + 1):
            if (l1, l2) not in PATHS[l]:
                continue
            p_idx = PATHS[l].index((l1, l2))
            cg = CG[(l1, l2, l)]
            for a in range(2 * l1 + 1):
                for b in range(2 * l2 + 1):
                    for m3 in range(2 * l + 1):
                        c = cg[a, b, m3]
                        if abs(c) > 1e-12:
                            uses.setdefault((a, b), []).append((l, p_idx, m3, float(c)))

        entries = []
        done_ab = set()
        for (a, b) in sorted(uses.keys()):
            if (a, b) in done_ab:
                continue
            pa, pb = 2 * l1 - a, 2 * l2 - b
            self_paired = (pa, pb) == (a, b)
            if self_paired or (pa, pb) not in uses or not USE_PAIR:
                done_ab.add((a, b))
                prods = [(A_OFF[l1] + a, A_OFF[l2] + b)]
                singles = [(l, p, m3, 0, c) for (l, p, m3, c) in uses[(a, b)]]
                entries.append(dict(prods=prods, merges=[], singles=singles))
                continue
            # two-product orbit
            done_ab.add((a, b))
            done_ab.add((pa, pb))
            prods = [(A_OFF[l1] + a, A_OFF[l2] + b),
                     (A_OFF[l1] + pa, A_OFF[l2] + pb)]
            merges, singles = [], []
            emitted = set()
            for half, (aa, bb) in enumerate([(a, b), (pa, pb)]):
                for (l, p, m3, c) in uses[(aa, bb)]:
                    if (l, p, m3, half) in emitted:
                        continue
                    s = (-1) ** (l1 + l2 - l)
                    pm = 2 * l - m3
                    if s == 1 and pm != m3:
                        # partner term lives on the other half with same cg
                        oh = 1 - half
                        if m3 < pm:
                            merges.append((l, p, m3, pm, half, c))
                        else:
                            merges.append((l, p, pm, m3, oh, c))
                        emitted.add((l, p, m3, half))
                        emitted.add((l, p, pm, oh))
                    elif (l, p, m3, half) not in emitted:
                        singles.append((l, p, m3, half, c))
                        emitted.add((l, p, m3, half))
            # dedupe merges (each pair appears from both halves)
            merges = sorted(set(merges))
            entries.append(dict(prods=prods, merges=merges, singles=singles))
        sched[(l1, l2)] = entries
    return sched


SCHED = _build_schedule()

DT = mybir.dt.float16
NP_DT = np.float16

_BUILT = None


def _build():
    """Build the single-core SPMD Bass program (same on all 8 cores)."""
    nc = bacc.Bacc(None, target_bir_lowering=False)

    f1t_ext = nc.declare_dram_parameter("f1t", [NKC, P, A_TOT * NP_], DT, isOutput=False)
    f2t_ext = nc.declare_dram_parameter("f2t", [NKC, P, A_TOT * NP_], DT, isOutput=False)
    n_up_chunks = sum(CIN[l] for l in range(4)) // P       # 68
    up_ext = nc.declare_dram_parameter("up", [n_up_chunks, P, K], DT, isOutput=False)
    out_ext = [nc.declare_dram_parameter(f"outT{l}", [NKC, K, (2 * l + 1) * NP_],
                                         mybir.dt.float32, isOutput=True)
               for l in range(4)]

    up_chunk_base = {}
    acc_chunks = 0
    for l in range(4):
        up_chunk_base[l] = acc_chunks
        acc_chunks += CIN[l] // P

    # engine pickers: per-slice deterministic so an accumulation chain for a
    # given tp slice stays on one engine (cross-engine RAW hops stall the
    # in-order queues)
    def make_picker(frac):
        cnt = [0.0]

        def pick():
            cnt[0] += frac
            if cnt[0] >= 1.0:
                cnt[0] -= 1.0
                return nc.gpsimd
            return nc.vector
        return pick

    prod_engine = make_picker(GPS_PROD_FRAC)

    def slice_engine(l, p_idx, m3):
        h = (l * 131 + p_idx * 31 + m3 * 7) % 100
        return nc.gpsimd if h < GPS_SINGLE_FRAC * 100 else nc.vector

    merge_engine = make_picker(GPS_MERGE_FRAC)

    with tile.TileContext(nc) as tc:
        with (
            tc.tile_pool(name="inp", bufs=1) as inp_pool,
            tc.tile_pool(name="upp", bufs=1) as up_pool,
            tc.tile_pool(name="tp", bufs=1) as tp_pool,
            tc.tile_pool(name="prod2", bufs=49) as prod2_pool,
            tc.tile_pool(name="prod1", bufs=6) as prod1_pool,
            tc.tile_pool(name="stage", bufs=4) as stage_pool,
            tc.tile_pool(name="ps", bufs=8, space="PSUM") as ps_pool,
        ):
            loop_ctx = ExitStack()
            if KERNEL_LOOP > 1:
                loop_ctx.enter_context(tc.For_i(0, KERNEL_LOOP, 1))

            up_t = up_pool.tile([P, n_up_chunks * K], DT, tag="up")
            nc.sync.dma_start(out=up_t[:], in_=up_ext.rearrange("c p k -> p c k"))

            f1_t = [inp_pool.tile([P, A_TOT * NP_], DT, tag=f"f1_{kc}", name=f"f1_{kc}")
                    for kc in range(NKC)]
            f2_t = [inp_pool.tile([P, A_TOT * NP_], DT, tag=f"f2_{kc}", name=f"f2_{kc}")
                    for kc in range(NKC)]
            for kc in range(NKC):
                nc.sync.dma_start(out=f1_t[kc][:], in_=f1t_ext[kc])
                nc.sync.dma_start(out=f2_t[kc][:], in_=f2t_ext[kc])

            RT = {l: [(i * 512, min(512, (2 * l + 1) * NP_ - i * 512))
                      for i in range(((2 * l + 1) * NP_ + 511) // 512)]
                  for l in range(4)}

            pending_evicts = []

            def emit_matmuls(l, kc, tp_t):
                flush_evicts()
                npaths = len(PATHS[l])
                for jh in range(2):
                    for ri, (r0, rw) in enumerate(RT[l]):
                        ps = ps_pool.tile([P, rw], mybir.dt.float32, tag="ps",
                                          name=f"ps_{l}_{jh}_{ri}_{kc}")
                        for pc in range(npaths):
                            chunk = up_chunk_base[l] + pc * NKC + kc
                            lhsT = up_t[:, chunk * K + jh * P: chunk * K + jh * P + P]
                            rhs = tp_t[(l, pc)][:, r0:r0 + rw]
                            nc.tensor.matmul(ps[:], lhsT, rhs,
                                             start=(pc == 0), stop=(pc == npaths - 1))
                        pending_evicts.append((ps, l, jh, ri, r0, rw, kc))

            def flush_evicts():
                for ps, l, jh, ri, r0, rw, kc in pending_evicts:
                    st = stage_pool.tile([P, rw], mybir.dt.float32, tag="stage",
                                         name=f"st_{l}_{jh}_{ri}_{kc}")
                    nc.scalar.activation(
                        st[:], ps[:], mybir.ActivationFunctionType.Copy)
                    nc.sync.dma_start(
                        out=out_ext[l][kc, jh * P:(jh + 1) * P, r0:r0 + rw],
                        in_=st[:])
                pending_evicts.clear()

            for kc in range(NKC):
                tp_t = {}
                tp3 = {}
                written = set()
                for l in range(4):
                    for p_idx in range(len(PATHS[l])):
                        t = tp_pool.tile([P, (2 * l + 1) * NP_], DT,
                                         tag=f"tp_{l}_{p_idx}", name=f"tp_{l}_{p_idx}_{kc}")
                        tp_t[(l, p_idx)] = t
                        tp3[(l, p_idx)] = t[:].rearrange("p (m n) -> p m n", n=NP_)

                for ci, (l1, l2) in enumerate(COMBO_ORDER):
                    entry_tiles = []
                    for entry in SCHED[(l1, l2)]:
                        prods = entry["prods"]
                        if len(prods) == 2:
                            (ga, gb), (gpa, gpb) = prods
                            da, db = gpa - ga, gpb - gb
                            pt = prod2_pool.tile([P, 2 * NP_], DT, tag="prod2",
                                                name=f"pp_{kc}_{l1}{l2}_{ga}_{gb}")
                            p3 = pt[:].rearrange("p (g n) -> p g n", n=NP_)
                            if USE_PAIR_PROD and da != 0 and db != 0:
                                # one 2-point-AP multiply computes both halves
                                b1 = f1_t[kc][:]
                                b2 = f2_t[kc][:]
                                in0 = bass.AP(b1.tensor, b1.offset + ga * NP_,
                                              [list(b1.ap[0]), [da * NP_, 2], [1, NP_]])
                                in1 = bass.AP(b2.tensor, b2.offset + gb * NP_,
                                              [list(b2.ap[0]), [db * NP_, 2], [1, NP_]])
                                prod_engine().tensor_mul(p3[:, 0:2, :], in0, in1)
                            else:
                                for h, (xa, xb) in enumerate(prods):
                                    prod_engine().tensor_mul(
                                        pt[:, h * NP_:(h + 1) * NP_],
                                        f1_t[kc][:, xa * NP_:(xa + 1) * NP_],
                                        f2_t[kc][:, xb * NP_:(xb + 1) * NP_])
                            halves = [pt[:, 0:NP_], pt[:, NP_:2 * NP_]]
                        else:
                            ga, gb = prods[0]
                            pt = prod1_pool.tile([P, NP_], DT, tag="prod1",
                                                name=f"pp_{kc}_{l1}{l2}_{ga}_{gb}")
                            p3 = None
                            prod_engine().tensor_mul(
                                pt[:],
                                f1_t[kc][:, ga * NP_:(ga + 1) * NP_],
                                f2_t[kc][:, gb * NP_:(gb + 1) * NP_])
                            halves = [pt[:], pt[:]]
                        entry_tiles.append((entry, p3, halves))

                    for entry, p3, halves in entry_tiles:
                        for (l, p_idx, mlo, mhi, hlo, cgv) in entry["merges"]:
                            dm = mhi - mlo
                            dst = tp3[(l, p_idx)][:, mlo:mhi + 1:dm, :]
                            src = p3[:, 0:2, :] if hlo == 0 else p3[:, 1::-1, :]
                            klo, khi = (l, p_idx, mlo), (l, p_idx, mhi)
                            if klo not in written and khi not in written:
                                written.add(klo)
                                written.add(khi)
                                if USE_ACT_FIRST:
                                    nc.scalar.activation(
                                        dst, src, mybir.ActivationFunctionType.Copy,
                                        scale=cgv)
                                else:
                                    nc.vector.tensor_scalar(
                                        dst, src, cgv, None, mybir.AluOpType.mult)
                            elif klo in written and khi in written:
                                if merge_engine() is nc.gpsimd:
                                    tmp = prod2_pool.tile([P, 2 * NP_], DT, tag="tmp2",
                                                          name=f"tm_{kc}_{l}_{p_idx}_{mlo}_{mhi}")
                                    t3 = tmp[:].rearrange("p (g n) -> p g n", n=NP_)
                                    nc.vector.tensor_scalar(
                                        t3[:, 0:2, :], src, cgv, None,
                                        mybir.AluOpType.mult)
                                    nc.gpsimd.tensor_add(dst, t3[:, 0:2, :], dst)
                                else:
                                    nc.vector.scalar_tensor_tensor(
                                        dst, src, cgv, dst,
                                        mybir.AluOpType.mult, mybir.AluOpType.add)
                            else:
                                for key, m3, h in ((klo, mlo, hlo), (khi, mhi, 1 - hlo)):
                                    d1 = tp3[(l, p_idx)][:, m3:m3 + 1, :]
                                    if key not in written:
                                        written.add(key)
                                        nc.vector.tensor_scalar(
                                            d1, p3[:, h:h + 1, :], cgv, None,
                                            mybir.AluOpType.mult)
                                    else:
                                        slice_engine(l, p_idx, m3).scalar_tensor_tensor(
                                            d1, p3[:, h:h + 1, :], cgv, d1,
                                            mybir.AluOpType.mult, mybir.AluOpType.add)

                        for (l, p_idx, m3, h, cgv) in entry["singles"]:
                            key = (l, p_idx, m3)
                            dst = tp3[(l, p_idx)][:, m3:m3 + 1, :]
                            src1 = halves[h].rearrange("p (g n) -> p g n", g=1)
                            if key not in written:
                                written.add(key)
                                if USE_ACT_FIRST:
                                    nc.scalar.activation(
                                        dst, src1,
                                        mybir.ActivationFunctionType.Copy, scale=cgv)
                                else:
                                    nc.vector.tensor_scalar(
                                        dst, src1, cgv, None, mybir.AluOpType.mult)
                            else:
                                slice_engine(l, p_idx, m3).scalar_tensor_tensor(
                                    dst, src1, cgv, dst,
                                    mybir.AluOpType.mult, mybir.AluOpType.add)

                    for l, ready in L_READY_AT.items():
                        if ci == ready:
                            emit_matmuls(l, kc, tp_t)
                flush_evicts()
            loop_ctx.close()


    nc.finalize()
    return nc


def _get_nc():
    global _BUILT
    if _BUILT is None:
        _BUILT = _build()
    return _BUILT


def _prep_in_maps(inputs):
    f1 = [np.asarray(inputs[f"f1_l{l}"], dtype=np.float32) for l in range(4)]
    U = [np.asarray(inputs[f"U_{l}"], dtype=np.float32) for l in range(4)]
    W = [np.asarray(inputs[f"W_{l}"], dtype=np.float32) for l in range(4)]
    up = np.concatenate(
        [(U[l].astype(np.float64) @ W[l].astype(np.float64)) for l in range(4)],
        axis=0)
    up_dev = np.ascontiguousarray(up.reshape(-1, P, K)).astype(NP_DT)

    f1_all = np.concatenate(f1, axis=1)
    f2_all = np.concatenate(
        [np.asarray(inputs[f"f2_l{l}"], dtype=np.float32) for l in range(4)], axis=1)

    in_maps = []
    for c in range(NCORES):
        sl = slice(c * NP_, (c + 1) * NP_)
        f1t = np.ascontiguousarray(f1_all[sl].transpose(2, 1, 0)).reshape(
            NKC, P, A_TOT * NP_).astype(NP_DT)
        f2t = np.ascontiguousarray(f2_all[sl].transpose(2, 1, 0)).reshape(
            NKC, P, A_TOT * NP_).astype(NP_DT)
        in_maps.append({"f1t": f1t, "f2t": f2t, "up": up_dev})
    return f1, in_maps


def kernel(**inputs):
    f1, in_maps = _prep_in_maps(inputs)
    res = run_bass_kernel_spmd(_get_nc(), in_maps, list(range(NCORES)))

    outs = []
    for l in range(4):
        pieces = []
        for c in range(NCORES):
            part = res.results[c][f"outT{l}"]
            mixed = (part[0] + part[1]).reshape(K, 2 * l + 1, NP_)
            pieces.append(mixed.transpose(2, 1, 0))
        mixed_full = np.concatenate(pieces, axis=0)
        outs.append((f1[l] + mixed_full).astype(np.float32))
    return tuple(outs)


# revision 37
# speedup vs baseline: 1.0198x; 1.0023x over previous
"""Trainium2 Bass kernel for the CG tensor-product iteration (nn_CGIteration).

Computation per l (L_MAX=3, K=256, N=2048):
    tp_l   = concat_{(l1,l2) in PATHS[l]} einsum('abm,nak,nbk->nmk', CG, f1_l1, f2_l2)
    out_l  = f1_l + (tp_l @ U_l) @ W_l

Sharding: data-parallel over N across 8 cores (N/8 = 256 per core).

Device-side layout: channels-on-partitions.  Host pre-transposes f1/f2 to
[k, a, n] per core, precomputes U'_l = U_l @ W_l (fp64) and the residual add +
final transpose happen on host.  On-chip per core:
  1. products P_ab[k, n] = f1T[:, a, :] * f2T[:, b, :]          (DVE/GpSimd, fp16)
  2. CG FMAs  tpT[(p,k), (m, n)] += cg * P_ab                   (DVE + ACT, fp16)
     - first write to a tp slice is a scaled copy on the Scalar engine
     - (m3, -m3) partner terms of even paths merge into one 2-point op
  3. U' matmul: psum[j, (m,n)] += U'[(p,k), j].T @ tpT           (PE, fp16 -> fp32)
     The two k-half phases are written to DRAM as separate fp32 partials
     and summed on the host together with the residual.
"""

import os
from contextlib import ExitStack
from math import factorial

import numpy as np

import concourse.bass as bass
import concourse.mybir as mybir
from concourse import bacc, tile
from concourse.bass_utils import run_bass_kernel_spmd

L_MAX = 3
K = 256
N = 2048
NCORES = 8
NP_ = N // NCORES          # samples per core (256)
P = 128                    # SBUF partitions
NKC = K // P               # k-half chunks (2)
A_OFF = [0, 1, 4, 9]       # offset of each l-block among the 16 m-components
A_TOT = 16

PATHS = {l: [(l1, l2) for l1 in range(L_MAX + 1) for l2 in range(L_MAX + 1)
             if abs(l1 - l2) <= l <= l1 + l2] for l in range(L_MAX + 1)}
CIN = {l: len(PATHS[l]) * K for l in range(L_MAX + 1)}

# fraction of ops routed to GpSimd (DVE is the bottleneck engine)
GPS_PROD_FRAC = float(os.environ.get("GPS_PROD_FRAC", "0.55"))
GPS_SINGLE_FRAC = float(os.environ.get("GPS_SINGLE_FRAC", "0.85"))
GPS_MERGE_FRAC = float(os.environ.get("GPS_MERGE_FRAC", "0.0"))
USE_PAIR = os.environ.get("PAIR", "1") == "1"
USE_ACT_FIRST = os.environ.get("ACT_FIRST", "1") == "1"
USE_PAIR_PROD = os.environ.get("PAIR_PROD", "1") == "1"
KERNEL_LOOP = int(os.environ.get("KERNEL_LOOP", "1"))


def _cg_coeff(l1, m1, l2, m2, l3, m3):
    if m3 != m1 + m2:
        return 0.0
    pref = ((2 * l3 + 1) * factorial(l3 + l1 - l2) * factorial(l3 - l1 + l2)
            * factorial(l1 + l2 - l3) / factorial(l1 + l2 + l3 + 1)) ** 0.5
    pref *= (factorial(l3 + m3) * factorial(l3 - m3) * factorial(l1 - m1)
             * factorial(l1 + m1) * factorial(l2 - m2) * factorial(l2 + m2)) ** 0.5
    s = 0.0
    for k in range(0, l1 + l2 - l3 + 1):
        d = [k, l1 + l2 - l3 - k, l1 - m1 - k, l2 + m2 - k,
             l3 - l2 + m1 + k, l3 - l1 - m2 + k]
        if min(d) < 0:
            continue
        den = 1.0
        for x in d:
            den *= factorial(x)
        s += (-1.0) ** k / den
    return pref * s


def _cg_tensor(l1, l2, l3):
    out = np.zeros((2 * l1 + 1, 2 * l2 + 1, 2 * l3 + 1), dtype=np.float64)
    for m1 in range(-l1, l1 + 1):
        for m2 in range(-l2, l2 + 1):
            m3 = m1 + m2
            if -l3 <= m3 <= l3:
                out[m1 + l1, m2 + l2, m3 + l3] = _cg_coeff(l1, m1, l2, m2, l3, m3)
    return out


CG = {(l1, l2, l): _cg_tensor(l1, l2, l)
      for l in range(L_MAX + 1) for (l1, l2) in PATHS[l]}

# combo order: complete the heaviest output l first so its matmuls overlap
# with the remaining CG work
COMBO_ORDER = ([(0, 3), (1, 2), (1, 3), (2, 1), (2, 2), (2, 3), (3, 0), (3, 1),
                (3, 2), (3, 3)] + [(0, 2), (1, 1), (2, 0)] + [(0, 1), (1, 0)]
               + [(0, 0)])
# after which combo (index) each l's paths are all complete
L_READY_AT = {3: 9, 2: 12, 1: 14, 0: 15}

# ---------------------------------------------------------------------------
# Static op schedule.  Per combo: a list of "orbit" entries.  Each entry:
#   prods: [(ga, gb)] or [(ga, gb), (gpa, gpb)]   global m-component indices
#   merges: [(l, p_idx, mlo, mhi, half_of_mlo, cg)]  2-point ops (mhi > mlo)
#   singles: [(l, p_idx, m3, half, cg)]
# ---------------------------------------------------------------------------


def _build_schedule():
    sched = {}
    for (l1, l2) in COMBO_ORDER:
        # gather all nonzero CG terms of this combo: (a, b) -> [(l, p, m3, cg)]
        uses = {}
        for l in range(L_MAX + 1):
            if (l1, l2) not in PATHS[l]:
                continue
            p_idx = PATHS[l].index((l1, l2))
            cg = CG[(l1, l2, l)]
            for a in range(2 * l1 + 1):
                for b in range(2 * l2 + 1):
                    for m3 in range(2 * l + 1):
                        c = cg[a, b, m3]
                        if abs(c) > 1e-12:
                            uses.setdefault((a, b), []).append((l, p_idx, m3, float(c)))

        entries = []
        done_ab = set()
        for (a, b) in sorted(uses.keys()):
            if (a, b) in done_ab:
                continue
            pa, pb = 2 * l1 - a, 2 * l2 - b
            self_paired = (pa, pb) == (a, b)
            if self_paired or (pa, pb) not in uses or not USE_PAIR:
                done_ab.add((a, b))
                prods = [(A_OFF[l1] + a, A_OFF[l2] + b)]
                singles = [(l, p, m3, 0, c) for (l, p, m3, c) in uses[(a, b)]]
                entries.append(dict(prods=prods, merges=[], singles=singles))
                continue
            # two-product orbit
            done_ab.add((a, b))
            done_ab.add((pa, pb))
            prods = [(A_OFF[l1] + a, A_OFF[l2] + b),
                     (A_OFF[l1] + pa, A_OFF[l2] + pb)]
            merges, singles = [], []
            emitted = set()
            for half, (aa, bb) in enumerate([(a, b), (pa, pb)]):
                for (l, p, m3, c) in uses[(aa, bb)]:
                    if (l, p, m3, half) in emitted:
                        continue
                    s = (-1) ** (l1 + l2 - l)
                    pm = 2 * l - m3
                    if s == 1 and pm != m3:
                        # partner term lives on the other half with same cg
                        oh = 1 - half
                        if m3 < pm:
                            merges.append((l, p, m3, pm, half, c))
                        else:
                            merges.append((l, p, pm, m3, oh, c))
                        emitted.add((l, p, m3, half))
                        emitted.add((l, p, pm, oh))
                    elif (l, p, m3, half) not in emitted:
                        singles.append((l, p, m3, half, c))
                        emitted.add((l, p, m3, half))
            # dedupe merges (each pair appears from both halves)
            merges = sorted(set(merges))
            entries.append(dict(prods=prods, merges=merges, singles=singles))
        sched[(l1, l2)] = entries
    return sched


SCHED = _build_schedule()

DT = mybir.dt.float16
NP_DT = np.float16

_BUILT = None


def _build():
    """Build the single-core SPMD Bass program (same on all 8 cores)."""
    nc = bacc.Bacc(None, target_bir_lowering=False)

    f1t_ext = nc.declare_dram_parameter("f1t", [NKC, P, A_TOT * NP_], DT, isOutput=False)
    f2t_ext = nc.declare_dram_parameter("f2t", [NKC, P, A_TOT * NP_], DT, isOutput=False)
    n_up_chunks = sum(CIN[l] for l in range(4)) // P       # 68
    up_ext = nc.declare_dram_parameter("up", [n_up_chunks, P, K], DT, isOutput=False)
    out_ext = [nc.declare_dram_parameter(f"outT{l}", [NKC, K, (2 * l + 1) * NP_],
                                         mybir.dt.float32, isOutput=True)
               for l in range(4)]

    up_chunk_base = {}
    acc_chunks = 0
    for l in range(4):
        up_chunk_base[l] = acc_chunks
        acc_chunks += CIN[l] // P

    # engine pickers: per-slice deterministic so an accumulation chain for a
    # given tp slice stays on one engine (cross-engine RAW hops stall the
    # in-order queues)
    def make_picker(frac):
        cnt = [0.0]

        def pick():
            cnt[0] += frac
            if cnt[0] >= 1.0:
                cnt[0] -= 1.0
                return nc.gpsimd
            return nc.vector
        return pick

    prod_engine = make_picker(GPS_PROD_FRAC)

    def slice_engine(l, p_idx, m3):
        h = (l * 131 + p_idx * 31 + m3 * 7) % 100
        return nc.gpsimd if h < GPS_SINGLE_FRAC * 100 else nc.vector

    merge_engine = make_picker(GPS_MERGE_FRAC)

    with tile.TileContext(nc) as tc:
        with (
            tc.tile_pool(name="inp", bufs=1) as inp_pool,
            tc.tile_pool(name="upp", bufs=1) as up_pool,
            tc.tile_pool(name="tp", bufs=1) as tp_pool,
            tc.tile_pool(name="prod2", bufs=52) as prod2_pool,
            tc.tile_pool(name="prod1", bufs=6) as prod1_pool,
            tc.tile_pool(name="stage", bufs=4) as stage_pool,
            tc.tile_pool(name="ps", bufs=8, space="PSUM") as ps_pool,
        ):
            loop_ctx = ExitStack()
            if KERNEL_LOOP > 1:
                loop_ctx.enter_context(tc.For_i(0, KERNEL_LOOP, 1))

            up_t = up_pool.tile([P, n_up_chunks * K], DT, tag="up")
            nc.sync.dma_start(out=up_t[:], in_=up_ext.rearrange("c p k -> p c k"))

            f1_t = [inp_pool.tile([P, A_TOT * NP_], DT, tag=f"f1_{kc}", name=f"f1_{kc}")
                    for kc in range(NKC)]
            f2_t = [inp_pool.tile([P, A_TOT * NP_], DT, tag=f"f2_{kc}", name=f"f2_{kc}")
                    for kc in range(NKC)]
            for kc in range(NKC):
                nc.sync.dma_start(out=f1_t[kc][:], in_=f1t_ext[kc])
                nc.sync.dma_start(out=f2_t[kc][:], in_=f2t_ext[kc])

            RT = {l: [(i * 512, min(512, (2 * l + 1) * NP_ - i * 512))
                      for i in range(((2 * l + 1) * NP_ + 511) // 512)]
                  for l in range(4)}

            pending_evicts = []

            def emit_matmuls(l, kc, tp_t):
                flush_evicts()
                npaths = len(PATHS[l])
                for jh in range(2):
                    for ri, (r0, rw) in enumerate(RT[l]):
                        ps = ps_pool.tile([P, rw], mybir.dt.float32, tag="ps",
                                          name=f"ps_{l}_{jh}_{ri}_{kc}")
                        for pc in range(npaths):
                            chunk = up_chunk_base[l] + pc * NKC + kc
                            lhsT = up_t[:, chunk * K + jh * P: chunk * K + jh * P + P]
                            rhs = tp_t[(l, pc)][:, r0:r0 + rw]
                            nc.tensor.matmul(ps[:], lhsT, rhs,
                                             start=(pc == 0), stop=(pc == npaths - 1))
                        pending_evicts.append((ps, l, jh, ri, r0, rw, kc))

            def flush_evicts():
                for ps, l, jh, ri, r0, rw, kc in pending_evicts:
                    st = stage_pool.tile([P, rw], mybir.dt.float32, tag="stage",
                                         name=f"st_{l}_{jh}_{ri}_{kc}")
                    nc.scalar.activation(
                        st[:], ps[:], mybir.ActivationFunctionType.Copy)
                    nc.sync.dma_start(
                        out=out_ext[l][kc, jh * P:(jh + 1) * P, r0:r0 + rw],
                        in_=st[:])
                pending_evicts.clear()

            for kc in range(NKC):
                tp_t = {}
                tp3 = {}
                written = set()
                for l in range(4):
                    for p_idx in range(len(PATHS[l])):
                        t = tp_pool.tile([P, (2 * l + 1) * NP_], DT,
                                         tag=f"tp_{l}_{p_idx}", name=f"tp_{l}_{p_idx}_{kc}")
                        tp_t[(l, p_idx)] = t
                        tp3[(l, p_idx)] = t[:].rearrange("p (m n) -> p m n", n=NP_)

                for ci, (l1, l2) in enumerate(COMBO_ORDER):
                    entry_tiles = []
                    for entry in SCHED[(l1, l2)]:
                        prods = entry["prods"]
                        if len(prods) == 2:
                            (ga, gb), (gpa, gpb) = prods
                            da, db = gpa - ga, gpb - gb
                            pt = prod2_pool.tile([P, 2 * NP_], DT, tag="prod2",
                                                name=f"pp_{kc}_{l1}{l2}_{ga}_{gb}")
                            p3 = pt[:].rearrange("p (g n) -> p g n", n=NP_)
                            if USE_PAIR_PROD and da != 0 and db != 0:
                                # one 2-point-AP multiply computes both halves
                                b1 = f1_t[kc][:]
                                b2 = f2_t[kc][:]
                                in0 = bass.AP(b1.tensor, b1.offset + ga * NP_,
                                              [list(b1.ap[0]), [da * NP_, 2], [1, NP_]])
                                in1 = bass.AP(b2.tensor, b2.offset + gb * NP_,
                                              [list(b2.ap[0]), [db * NP_, 2], [1, NP_]])
                                prod_engine().tensor_mul(p3[:, 0:2, :], in0, in1)
                            else:
                                for h, (xa, xb) in enumerate(prods):
                                    prod_engine().tensor_mul(
                                        pt[:, h * NP_:(h + 1) * NP_],
                                        f1_t[kc][:, xa * NP_:(xa + 1) * NP_],
                                        f2_t[kc][:, xb * NP_:(xb + 1) * NP_])
                            halves = [pt[:, 0:NP_], pt[:, NP_:2 * NP_]]
                        else:
                            ga, gb = prods[0]
                            pt = prod1_pool.tile([P, NP_], DT, tag="prod1",
                                                name=f"pp_{kc}_{l1}{l2}_{ga}_{gb}")
                            p3 = None
                            prod_engine().tensor_mul(
                                pt[:],
                                f1_t[kc][:, ga * NP_:(ga + 1) * NP_],
                                f2_t[kc][:, gb * NP_:(gb + 1) * NP_])
                            halves = [pt[:], pt[:]]
                        entry_tiles.append((entry, p3, halves))

                    for entry, p3, halves in entry_tiles:
                        for (l, p_idx, mlo, mhi, hlo, cgv) in entry["merges"]:
                            dm = mhi - mlo
                            dst = tp3[(l, p_idx)][:, mlo:mhi + 1:dm, :]
                            src = p3[:, 0:2, :] if hlo == 0 else p3[:, 1::-1, :]
                            klo, khi = (l, p_idx, mlo), (l, p_idx, mhi)
                            if klo not in written and khi not in written:
                                written.add(klo)
                                written.add(khi)
                                if USE_ACT_FIRST:
                                    nc.scalar.activation(
                                        dst, src, mybir.ActivationFunctionType.Copy,
                                        scale=cgv)
                                else:
                                    nc.vector.tensor_scalar(
                                        dst, src, cgv, None, mybir.AluOpType.mult)
                            elif klo in written and khi in written:
                                if merge_engine() is nc.gpsimd:
                                    tmp = prod2_pool.tile([P, 2 * NP_], DT, tag="tmp2",
                                                          name=f"tm_{kc}_{l}_{p_idx}_{mlo}_{mhi}")
                                    t3 = tmp[:].rearrange("p (g n) -> p g n", n=NP_)
                                    nc.vector.tensor_scalar(
                                        t3[:, 0:2, :], src, cgv, None,
                                        mybir.AluOpType.mult)
                                    nc.gpsimd.tensor_add(dst, t3[:, 0:2, :], dst)
                                else:
                                    nc.vector.scalar_tensor_tensor(
                                        dst, src, cgv, dst,
                                        mybir.AluOpType.mult, mybir.AluOpType.add)
                            else:
                                for key, m3, h in ((klo, mlo, hlo), (khi, mhi, 1 - hlo)):
                                    d1 = tp3[(l, p_idx)][:, m3:m3 + 1, :]
                                    if key not in written:
                                        written.add(key)
                                        nc.vector.tensor_scalar(
                                            d1, p3[:, h:h + 1, :], cgv, None,
                                            mybir.AluOpType.mult)
                                    else:
                                        slice_engine(l, p_idx, m3).scalar_tensor_tensor(
                                            d1, p3[:, h:h + 1, :], cgv, d1,
                                            mybir.AluOpType.mult, mybir.AluOpType.add)

                        for (l, p_idx, m3, h, cgv) in entry["singles"]:
                            key = (l, p_idx, m3)
                            dst = tp3[(l, p_idx)][:, m3:m3 + 1, :]
                            src1 = halves[h].rearrange("p (g n) -> p g n", g=1)
                            if key not in written:
                                written.add(key)
                                if USE_ACT_FIRST:
                                    nc.scalar.activation(
                                        dst, src1,
                                        mybir.ActivationFunctionType.Copy, scale=cgv)
                                else:
                                    nc.vector.tensor_scalar(
                                        dst, src1, cgv, None, mybir.AluOpType.mult)
                            else:
                                slice_engine(l, p_idx, m3).scalar_tensor_tensor(
                                    dst, src1, cgv, dst,
                                    mybir.AluOpType.mult, mybir.AluOpType.add)

                    for l, ready in L_READY_AT.items():
                        if ci == ready:
                            emit_matmuls(l, kc, tp_t)
                flush_evicts()
            loop_ctx.close()


    nc.finalize()
    return nc


def _get_nc():
    global _BUILT
    if _BUILT is None:
        _BUILT = _build()
    return _BUILT


def _prep_in_maps(inputs):
    f1 = [np.asarray(inputs[f"f1_l{l}"], dtype=np.float32) for l in range(4)]
    U = [np.asarray(inputs[f"U_{l}"], dtype=np.float32) for l in range(4)]
    W = [np.asarray(inputs[f"W_{l}"], dtype=np.float32) for l in range(4)]
    up = np.concatenate(
        [(U[l].astype(np.float64) @ W[l].astype(np.float64)) for l in range(4)],
        axis=0)
    up_dev = np.ascontiguousarray(up.reshape(-1, P, K)).astype(NP_DT)

    f1_all = np.concatenate(f1, axis=1)
    f2_all = np.concatenate(
        [np.asarray(inputs[f"f2_l{l}"], dtype=np.float32) for l in range(4)], axis=1)

    in_maps = []
    for c in range(NCORES):
        sl = slice(c * NP_, (c + 1) * NP_)
        f1t = np.ascontiguousarray(f1_all[sl].transpose(2, 1, 0)).reshape(
            NKC, P, A_TOT * NP_).astype(NP_DT)
        f2t = np.ascontiguousarray(f2_all[sl].transpose(2, 1, 0)).reshape(
            NKC, P, A_TOT * NP_).astype(NP_DT)
        in_maps.append({"f1t": f1t, "f2t": f2t, "up": up_dev})
    return f1, in_maps


def kernel(**inputs):
    f1, in_maps = _prep_in_maps(inputs)
    res = run_bass_kernel_spmd(_get_nc(), in_maps, list(range(NCORES)))

    outs = []
    for l in range(4):
        pieces = []
        for c in range(NCORES):
            part = res.results[c][f"outT{l}"]
            mixed = (part[0] + part[1]).reshape(K, 2 * l + 1, NP_)
            pieces.append(mixed.transpose(2, 1, 0))
        mixed_full = np.concatenate(pieces, axis=0)
        outs.append((f1[l] + mixed_full).astype(np.float32))
    return tuple(outs)
